# revision 6
# baseline (speedup 1.0000x reference)
"""Multi-head self-attention + vocab projection, 8-core TRN2 Bass kernel.

Problem: x[2,2048,1024] -> logits[2,2048,32000]
  q/k/v = x@W{q,k,v}+b, 16 heads x 64; attn = softmax(qk^T/8)v; out = attn@Wo+bo

Sharding: data-parallel over the 4096 token rows -> 8 cores x 512 query rows
(cores 0-3 batch 0, cores 4-7 batch 1). Each core receives its full batch
(2048 tokens) for K/V, ROLLED so that its 512 query rows are rows 0:512 —
softmax is permutation-invariant over the kv axis, so rolling is safe and
makes the SPMD program core-id independent. Wo is column-streamed in full on
every core; logits are written with no cross-core reduce.

All heavy matmuls run in float32r (fp32-precision tensor-engine mode, 1
cycle/row for free-dim >= 256). Attention operands (qT/kT/v/exp-scores) are
stored bf16 to fit SBUF. Attention is computed transposed
(scoresT[j,q] = kT^T qT) so exp(scoresT) feeds attn@V directly as lhsT and
the attention output lands as attn_outT[emb, tok] — exactly the lhsT layout
the vocab projection needs. The softmax denominator comes from an appended
ones-column on V; normalization is applied after attn@V via a
reciprocal + DRAM-broadcast + elementwise multiply.
"""

import numpy as np

B, S, E = 2, 2048, 1024
H, D = 16, 64
V = 32000
P = 128
ET = E // P          # 8 embedding tiles
TOK = S              # kv tokens per core
Q = 512              # query rows per core
NJT = TOK // P       # 16 kv tiles
VCH = 500            # vocab chunk (psum bank = 512 f32)
NVC = V // VCH       # 64
DVC = 256            # v-projection dout chunk (= 4 heads)
NCORES = 8

_cache = {}


def _build():
    from contextlib import ExitStack

    import concourse.tile as tile
    from concourse import bacc, mybir
    from concourse.masks import make_identity

    f32 = mybir.dt.float32
    f32r = mybir.dt.float32r
    bf16 = mybir.dt.bfloat16
    Id = mybir.ActivationFunctionType.Identity
    Exp = mybir.ActivationFunctionType.Exp

    nc = bacc.Bacc("TRN2", target_bir_lowering=False, debug=False,
                   num_devices=NCORES)

    xb = nc.dram_tensor("xb", [TOK, E], f32, kind="ExternalInput").ap()
    wq = nc.dram_tensor("wq", [E, E], f32r, kind="ExternalInput").ap()
    wk = nc.dram_tensor("wk", [E, E], f32r, kind="ExternalInput").ap()
    wv = nc.dram_tensor("wv", [E, E], f32r, kind="ExternalInput").ap()
    wo = nc.dram_tensor("wo", [E, V], f32r, kind="ExternalInput").ap()
    bq = nc.dram_tensor("bq", [E], f32, kind="ExternalInput").ap()
    bk = nc.dram_tensor("bk", [E], f32, kind="ExternalInput").ap()
    bv = nc.dram_tensor("bv", [E], f32r, kind="ExternalInput").ap()
    bo = nc.dram_tensor("bo", [V], f32r, kind="ExternalInput").ap()
    out = nc.dram_tensor("out", [Q, V], f32, kind="ExternalOutput").ap()

    wq3 = wq.rearrange("(et p) d -> p et d", p=P)
    wk3 = wk.rearrange("(et p) d -> p et d", p=P)
    wv3 = wv.rearrange("(et p) d -> p et d", p=P)
    wo3 = wo.rearrange("(et p) v -> p et v", p=P)

    with tile.TileContext(nc) as tc, ExitStack() as ctx:
        # ---- long-lived pools ------------------------------------------
        consts = ctx.enter_context(tc.tile_pool(name="consts", bufs=1))
        kT_pool = ctx.enter_context(tc.tile_pool(name="kT", bufs=1))
        vA_pool = ctx.enter_context(tc.tile_pool(name="vA", bufs=1))
        qT_pool = ctx.enter_context(tc.tile_pool(name="qT", bufs=1))
        dn_pool = ctx.enter_context(tc.tile_pool(name="dn", bufs=1))
        dram_pool = ctx.enter_context(
            tc.tile_pool(name="dramscratch", bufs=1, space="DRAM"))

        ident = consts.tile([P, P], f32)
        make_identity(nc, ident)
        bq_sb = consts.tile([P, ET], f32)
        nc.sync.dma_start(bq_sb[:], bq.rearrange("(g p) -> p g", p=P))
        # fold the 1/sqrt(d) score scale into q: q' = (xWq + bq)/8
        nc.vector.tensor_scalar_mul(bq_sb[:], bq_sb[:], 0.125)
        bk_sb = consts.tile([P, ET], f32)
        nc.sync.dma_start(bk_sb[:], bk.rearrange("(g p) -> p g", p=P))
        bv_row = consts.tile([1, E], f32r)
        nc.sync.dma_start(bv_row[:], bv.rearrange("(o e) -> o e", o=1))
        ones_f = consts.tile([1, P], f32)
        nc.vector.memset(ones_f[:], 1.0)
        ones_r = consts.tile([1, P], f32r)
        nc.vector.tensor_copy(ones_r[:], ones_f[:])

        kT = kT_pool.tile([P, ET, TOK], bf16)    # k^T, d on partitions
        vA = vA_pool.tile([P, NJT, H * 65], bf16)  # v + ones col, per kv tile
        qT = qT_pool.tile([P, ET, Q], bf16)      # (q/8)^T
        denoms = dn_pool.tile([H, Q], f32)
        recips = dn_pool.tile([H, Q], f32)
        den_dram = dram_pool.tile([H, Q], f32)
        rec_dram = dram_pool.tile([H, Q], f32)

        # ones column of vA (head-local column 64)
        vA4 = vA[:].rearrange("p j (h c) -> p j h c", c=65)
        nc.vector.memset(vA4[:, :, :, 64:65], 1.0)

        # ---- phase 1+2: transpose x, project q/k/v (scoped pools) ------
        with ExitStack() as pctx:
            xT_pool = pctx.enter_context(tc.tile_pool(name="xT", bufs=1))
            xload = pctx.enter_context(tc.tile_pool(name="xload", bufs=3))
            wqk_pool = pctx.enter_context(tc.tile_pool(name="wqk", bufs=2))
            wv_pool = pctx.enter_context(tc.tile_pool(name="wvp", bufs=2))
            psT = pctx.enter_context(
                tc.tile_pool(name="psT", bufs=4, space="PSUM"))
            psP = pctx.enter_context(
                tc.tile_pool(name="psP", bufs=2, space="PSUM"))
            psV = pctx.enter_context(
                tc.tile_pool(name="psV", bufs=2, space="PSUM"))

            xT = xT_pool.tile([P, ET, TOK], f32r)  # x^T, emb on partitions

            for tt in range(NJT):
                xt = xload.tile([P, E], f32)
                nc.sync.dma_start(xt[:], xb[tt * P:(tt + 1) * P, :])
                for et in range(ET):
                    pst = psT.tile([P, P], f32)
                    nc.tensor.transpose(pst[:], xt[:, et * P:(et + 1) * P],
                                        ident[:])
                    dst = xT[:, et, tt * P:(tt + 1) * P]
                    if et % 2 == 0:
                        nc.vector.tensor_copy(dst, pst[:])
                    else:
                        nc.scalar.copy(dst, pst[:])

            # k^T and q^T: out[dout_tile, tok] = Wx^T(emb,dout)^T @ x^T
            for w3, dest, bias_sb, scale, ntc in (
                (wk3, kT, bk_sb, 1.0, TOK // 512),
                (wq3, qT, bq_sb, 0.125, Q // 512),
            ):
                for dt in range(ET):
                    w_t = wqk_pool.tile([P, ET, P], f32r, tag="wqk")
                    nc.sync.dma_start(w_t[:], w3[:, :, dt * P:(dt + 1) * P])
                    for tcc in range(ntc):
                        ps = psP.tile([P, 512], f32)
                        for et in range(ET):
                            nc.tensor.matmul(
                                ps[:], w_t[:, et, :],
                                xT[:, et, tcc * 512:(tcc + 1) * 512],
                                start=(et == 0), stop=(et == ET - 1))
                        nc.scalar.activation(
                            dest[:, dt, tcc * 512:(tcc + 1) * 512], ps[:],
                            Id, bias=bias_sb[:, dt:dt + 1], scale=scale)

            # v (natural layout) + bias via K=1 ones-matmul
            for dvc in range(E // DVC):
                wv_t = wv_pool.tile([P, ET, DVC], f32r, tag="wv")
                nc.sync.dma_start(wv_t[:],
                                  wv3[:, :, dvc * DVC:(dvc + 1) * DVC])
                for tt in range(NJT):
                    ps = psV.tile([P, DVC], f32)
                    for et in range(ET):
                        nc.tensor.matmul(ps[:],
                                         xT[:, et, tt * P:(tt + 1) * P],
                                         wv_t[:, et, :],
                                         start=(et == 0), stop=False)
                    nc.tensor.matmul(
                        ps[:], ones_r[0:1, :],
                        bv_row[0:1, dvc * DVC:(dvc + 1) * DVC],
                        start=False, stop=True)
                    h0 = dvc * (DVC // D)
                    dst = vA4[:, tt, h0:h0 + DVC // D, 0:64]
                    nc.vector.tensor_copy(
                        dst, ps[:].rearrange("p (h c) -> p h c", c=D))

        # ---- phase 3: attention (transposed) ---------------------------
        aT_pool = ctx.enter_context(tc.tile_pool(name="aT", bufs=1))
        aT = aT_pool.tile([P, ET, Q], f32r)  # attn_out^T, emb on partitions

        with ExitStack() as pctx:
            e_pool = pctx.enter_context(tc.tile_pool(name="epool", bufs=6))
            den_pool = pctx.enter_context(tc.tile_pool(name="denrow",
                                                       bufs=2))
            psS = pctx.enter_context(
                tc.tile_pool(name="psS", bufs=2, space="PSUM"))
            psA = pctx.enter_context(
                tc.tile_pool(name="psA", bufs=2, space="PSUM"))

            for h in range(H):
                dt, dr = h // 2, (h % 2) * D
                po = psA.tile([P, Q], f32)
                for jt in range(NJT):
                    ps = psS.tile([P, Q], f32)
                    # scoresT[j, q] = kT(:,jtile)^T @ qT  (K = d = 64)
                    nc.tensor.matmul(ps[:],
                                     kT[dr:dr + D, dt, jt * P:(jt + 1) * P],
                                     qT[dr:dr + D, dt, :],
                                     start=True, stop=True)
                    e = e_pool.tile([P, Q], bf16, tag="e")
                    nc.scalar.activation(e[:], ps[:], Exp)
                    # attn_outT rows = [v | 1]^T @ E : row 64 = denom
                    nc.tensor.matmul(po[0:65, :],
                                     vA[:, jt, h * 65:(h + 1) * 65], e[:],
                                     start=(jt == 0), stop=(jt == NJT - 1))
                # denom row lives on psum partition 64; engines can't move
                # across partitions, so bounce via SBUF row 64 + DMA.
                den_t = den_pool.tile([P, Q], f32, tag="denrow")
                nc.scalar.copy(den_t[64:65, :], po[64:65, :])
                nc.sync.dma_start(den_dram[h:h + 1, :], den_t[64:65, :])
                nc.scalar.copy(aT[dr:dr + D, dt, :], po[0:64, :])

        # ---- phase 3.5: normalize by softmax denominator ---------------
        with ExitStack() as pctx:
            rbc_pool = pctx.enter_context(tc.tile_pool(name="rbc", bufs=2))
            nc.sync.dma_start(denoms[:], den_dram[:])
            nc.vector.reciprocal(recips[:], denoms[:])
            nc.sync.dma_start(rec_dram[:], recips[:])
            for g in range(ET):  # head pair (2g, 2g+1) shares partition tile g
                rbc = rbc_pool.tile([P, Q], f32, tag="rbc")
                nc.sync.dma_start(
                    rbc[0:D, :],
                    rec_dram[2 * g:2 * g + 1, :].to_broadcast((D, Q)))
                nc.sync.dma_start(
                    rbc[D:P, :],
                    rec_dram[2 * g + 1:2 * g + 2, :].to_broadcast((D, Q)))
                nc.vector.tensor_tensor(aT[:, g, :], aT[:, g, :], rbc[:],
                                        mybir.AluOpType.mult)

        # ---- phase 4: vocab projection ---------------------------------
        with ExitStack() as pctx:
            wo_pool = pctx.enter_context(tc.tile_pool(name="wo", bufs=3))
            lt_pool = pctx.enter_context(tc.tile_pool(name="lt", bufs=4))
            bo_pool = pctx.enter_context(tc.tile_pool(name="bo", bufs=2))
            psO = pctx.enter_context(
                tc.tile_pool(name="psO", bufs=4, space="PSUM"))

            for vc in range(NVC):
                wo_t = wo_pool.tile([P, ET, VCH], f32r, tag="wo")
                nc.sync.dma_start(wo_t[:],
                                  wo3[:, :, vc * VCH:(vc + 1) * VCH])
                bo_t = bo_pool.tile([1, VCH], f32r, tag="bo")
                nc.sync.dma_start(
                    bo_t[:],
                    bo[vc * VCH:(vc + 1) * VCH].rearrange("(o v) -> o v",
                                                          o=1))
                for tc4 in range(Q // P):
                    ps = psO.tile([P, VCH], f32)
                    for et in range(ET):
                        nc.tensor.matmul(ps[:],
                                         aT[:, et, tc4 * P:(tc4 + 1) * P],
                                         wo_t[:, et, :],
                                         start=(et == 0), stop=False)
                    nc.tensor.matmul(ps[:], ones_r[0:1, :], bo_t[0:1, :],
                                     start=False, stop=True)
                    lt = lt_pool.tile([P, VCH], f32, tag="lt")
                    nc.vector.tensor_copy(lt[:], ps[:])
                    nc.sync.dma_start(
                        out[tc4 * P:(tc4 + 1) * P, vc * VCH:(vc + 1) * VCH],
                        lt[:])

    nc.compile()
    return nc


def get_nc():
    if "nc" not in _cache:
        _cache["nc"] = _build()
    return _cache["nc"]


def make_in_maps(x, Wq, bq, Wk, bk, Wv, bv, Wo, bo):
    x = np.ascontiguousarray(np.asarray(x, dtype=np.float32))
    Wq = np.ascontiguousarray(np.asarray(Wq, dtype=np.float32))
    Wk = np.ascontiguousarray(np.asarray(Wk, dtype=np.float32))
    Wv = np.ascontiguousarray(np.asarray(Wv, dtype=np.float32))
    Wo = np.ascontiguousarray(np.asarray(Wo, dtype=np.float32))
    bq = np.ascontiguousarray(np.asarray(bq, dtype=np.float32))
    bk = np.ascontiguousarray(np.asarray(bk, dtype=np.float32))
    bv = np.ascontiguousarray(np.asarray(bv, dtype=np.float32))
    bo = np.ascontiguousarray(np.asarray(bo, dtype=np.float32))
    in_maps = []
    for c in range(NCORES):
        b, qs = c // 4, (c % 4) * Q
        xb = np.ascontiguousarray(np.roll(x[b], -qs, axis=0))
        in_maps.append({"xb": xb, "wq": Wq, "wk": Wk, "wv": Wv, "wo": Wo,
                        "bq": bq, "bk": bk, "bv": bv, "bo": bo})
    return in_maps


def gather(results):
    out = np.empty((B, S, V), dtype=np.float32)
    for c in range(NCORES):
        b, qs = c // 4, (c % 4) * Q
        out[b, qs:qs + Q] = results[c]["out"]
    return out


def kernel(**inputs):
    from concourse.bass_utils import run_bass_kernel_spmd

    nc = get_nc()
    in_maps = make_in_maps(**inputs)
    res = run_bass_kernel_spmd(nc, in_maps, list(range(NCORES)), trace=False)
    return gather(res.results)


# revision 7
# speedup vs baseline: 1.1321x; 1.1321x over previous
"""Multi-head self-attention + vocab projection, 8-core TRN2 Bass kernel.

Problem: x[2,2048,1024] -> logits[2,2048,32000]
  q/k/v = x@W{q,k,v}+b, 16 heads x 64; attn = softmax(qk^T/8)v; out = attn@Wo+bo

Sharding: data-parallel over the 4096 token rows -> 8 cores x 512 query rows
(cores 0-3 batch 0, cores 4-7 batch 1). Each core receives its full batch
(2048 tokens) for K/V, ROLLED so that its 512 query rows are rows 0:512 —
softmax is permutation-invariant over the kv axis, so rolling is safe and
makes the SPMD program core-id independent. Wo is column-streamed in full on
every core; logits are written with no cross-core reduce.

x and all weights are converted to bf16 host-side (halves HBM traffic for
the dominant Wo stream; matmuls accumulate in f32 psum). xT comes from XBAR
DMA-transpose (2-byte dtype) — no PE transposes. Attention is computed
transposed (scoresT[j,q] = kT^T qT) so exp(scoresT) feeds attn@V directly as
lhsT and the attention output lands as attn_outT[emb, tok] — exactly the
lhsT layout the vocab projection needs. The softmax denominator comes from
an appended ones-column on V; normalization is applied after attn@V via
reciprocal + DRAM-broadcast + elementwise multiply. bo is added during the
psum->sbuf copy against a DMA-broadcast bias tile.
"""

import numpy as np

B, S, E = 2, 2048, 1024
H, D = 16, 64
V = 32000
P = 128
ET = E // P          # 8 embedding tiles
TOK = S              # kv tokens per core
Q = 512              # query rows per core
NJT = TOK // P       # 16 kv tiles
VCH = 500            # vocab chunk (psum bank = 512 f32)
NVC = V // VCH       # 64
DVC = 256            # v-projection dout chunk (= 4 heads)
NCORES = 8

_cache = {}


def _build():
    from contextlib import ExitStack

    import concourse.tile as tile
    from concourse import bacc, mybir

    f32 = mybir.dt.float32
    bf16 = mybir.dt.bfloat16
    Id = mybir.ActivationFunctionType.Identity
    Exp = mybir.ActivationFunctionType.Exp

    nc = bacc.Bacc("TRN2", target_bir_lowering=False, debug=False,
                   num_devices=NCORES)

    xb = nc.dram_tensor("xb", [TOK, E], bf16, kind="ExternalInput").ap()
    wq = nc.dram_tensor("wq", [E, E], bf16, kind="ExternalInput").ap()
    wk = nc.dram_tensor("wk", [E, E], bf16, kind="ExternalInput").ap()
    wv = nc.dram_tensor("wv", [E, E], bf16, kind="ExternalInput").ap()
    wo = nc.dram_tensor("wo", [E, V], bf16, kind="ExternalInput").ap()
    bq = nc.dram_tensor("bq", [E], f32, kind="ExternalInput").ap()
    bk = nc.dram_tensor("bk", [E], f32, kind="ExternalInput").ap()
    bv = nc.dram_tensor("bv", [E], bf16, kind="ExternalInput").ap()
    bo = nc.dram_tensor("bo", [V], bf16, kind="ExternalInput").ap()
    out = nc.dram_tensor("out", [Q, V], f32, kind="ExternalOutput").ap()

    wq3 = wq.rearrange("(et p) d -> p et d", p=P)
    wk3 = wk.rearrange("(et p) d -> p et d", p=P)
    wv3 = wv.rearrange("(et p) d -> p et d", p=P)
    wo3 = wo.rearrange("(et p) v -> p et v", p=P)

    with tile.TileContext(nc) as tc, ExitStack() as ctx:
        # ---- pools (SBUF all top-level: lets DMA prefetch run ahead) ---
        consts = ctx.enter_context(tc.tile_pool(name="consts", bufs=1))
        xT_pool = ctx.enter_context(tc.tile_pool(name="xT", bufs=1))
        kT_pool = ctx.enter_context(tc.tile_pool(name="kT", bufs=1))
        vA_pool = ctx.enter_context(tc.tile_pool(name="vA", bufs=1))
        qT_pool = ctx.enter_context(tc.tile_pool(name="qT", bufs=1))
        aT_pool = ctx.enter_context(tc.tile_pool(name="aT", bufs=1))
        dn_pool = ctx.enter_context(tc.tile_pool(name="dn", bufs=1))
        wqk_pool = ctx.enter_context(tc.tile_pool(name="wqk", bufs=2))
        wv_pool = ctx.enter_context(tc.tile_pool(name="wvp", bufs=2))
        e_pool = ctx.enter_context(tc.tile_pool(name="epool", bufs=6))
        den_pool = ctx.enter_context(tc.tile_pool(name="denrow", bufs=2))
        rbc_pool = ctx.enter_context(tc.tile_pool(name="rbc", bufs=2))
        wo_pool = ctx.enter_context(tc.tile_pool(name="wo", bufs=4))
        lt_pool = ctx.enter_context(tc.tile_pool(name="lt", bufs=4))
        bo_pool = ctx.enter_context(tc.tile_pool(name="bo", bufs=3))
        dram_pool = ctx.enter_context(
            tc.tile_pool(name="dramscratch", bufs=1, space="DRAM"))
        psO = ctx.enter_context(tc.tile_pool(name="psO", bufs=4,
                                             space="PSUM"))

        bq_sb = consts.tile([P, ET], f32)
        nc.sync.dma_start(bq_sb[:], bq.rearrange("(g p) -> p g", p=P))
        # fold the 1/sqrt(d) score scale into q: q' = (xWq + bq)/8
        nc.vector.tensor_scalar_mul(bq_sb[:], bq_sb[:], 0.125)
        bk_sb = consts.tile([P, ET], f32)
        nc.sync.dma_start(bk_sb[:], bk.rearrange("(g p) -> p g", p=P))
        bv_row = consts.tile([1, E], bf16)
        nc.sync.dma_start(bv_row[:], bv.rearrange("(o e) -> o e", o=1))
        ones_b = consts.tile([1, P], bf16)
        nc.vector.memset(ones_b[:], 1.0)

        xT = xT_pool.tile([P, ET, TOK], bf16)    # x^T, emb on partitions
        kT = kT_pool.tile([P, ET, TOK], bf16)    # k^T, d on partitions
        vA = vA_pool.tile([P, NJT, H * 65], bf16)  # v + ones col, per kv tile
        qT = qT_pool.tile([P, ET, Q], bf16)      # (q/8)^T
        aT = aT_pool.tile([P, ET, Q], bf16)      # attn_out^T
        denoms = dn_pool.tile([H, Q], f32)
        recips = dn_pool.tile([H, Q], f32)
        den_dram = dram_pool.tile([H, Q], f32)
        rec_dram = dram_pool.tile([H, Q], f32)

        # ones column of vA (head-local column 64)
        vA4 = vA[:].rearrange("p j (h c) -> p j h c", c=65)
        nc.vector.memset(vA4[:, :, :, 64:65], 1.0)

        # ---- phase 1: x^T via XBAR dma-transpose -----------------------
        for et in range(ET):
            nc.sync.dma_start_transpose(xT[:, et, :],
                                        xb[:, et * P:(et + 1) * P])

        # ---- phase 2: q/k/v projections --------------------------------
        with ExitStack() as pctx:
            psP = pctx.enter_context(
                tc.tile_pool(name="psP", bufs=2, space="PSUM"))
            psV = pctx.enter_context(
                tc.tile_pool(name="psV", bufs=2, space="PSUM"))

            # k^T and q^T: out[dout_tile, tok] = W(emb,dout)^T @ x^T
            for w3, dest, bias_sb, scale, ntc in (
                (wk3, kT, bk_sb, 1.0, TOK // 512),
                (wq3, qT, bq_sb, 0.125, Q // 512),
            ):
                for dt in range(ET):
                    w_t = wqk_pool.tile([P, ET, P], bf16, tag="wqk")
                    nc.sync.dma_start(w_t[:], w3[:, :, dt * P:(dt + 1) * P])
                    for tcc in range(ntc):
                        ps = psP.tile([P, 512], f32)
                        for et in range(ET):
                            nc.tensor.matmul(
                                ps[:], w_t[:, et, :],
                                xT[:, et, tcc * 512:(tcc + 1) * 512],
                                start=(et == 0), stop=(et == ET - 1))
                        nc.scalar.activation(
                            dest[:, dt, tcc * 512:(tcc + 1) * 512], ps[:],
                            Id, bias=bias_sb[:, dt:dt + 1], scale=scale)

            # v (natural layout) + bias via K=1 ones-matmul
            for dvc in range(E // DVC):
                wv_t = wv_pool.tile([P, ET, DVC], bf16, tag="wv")
                nc.sync.dma_start(wv_t[:],
                                  wv3[:, :, dvc * DVC:(dvc + 1) * DVC])
                for tt in range(NJT):
                    ps = psV.tile([P, DVC], f32)
                    for et in range(ET):
                        nc.tensor.matmul(ps[:],
                                         xT[:, et, tt * P:(tt + 1) * P],
                                         wv_t[:, et, :],
                                         start=(et == 0), stop=False)
                    nc.tensor.matmul(
                        ps[:], ones_b[0:1, :],
                        bv_row[0:1, dvc * DVC:(dvc + 1) * DVC],
                        start=False, stop=True)
                    h0 = dvc * (DVC // D)
                    dst = vA4[:, tt, h0:h0 + DVC // D, 0:64]
                    nc.vector.tensor_copy(
                        dst, ps[:].rearrange("p (h c) -> p h c", c=D))

        # ---- phase 3: attention (transposed) ---------------------------
        with ExitStack() as pctx:
            psS = pctx.enter_context(
                tc.tile_pool(name="psS", bufs=2, space="PSUM"))
            psA = pctx.enter_context(
                tc.tile_pool(name="psA", bufs=2, space="PSUM"))

            for h in range(H):
                dt, dr = h // 2, (h % 2) * D
                po = psA.tile([P, Q], f32)
                for jt in range(NJT):
                    ps = psS.tile([P, Q], f32)
                    # scoresT[j, q] = kT(:,jtile)^T @ qT  (K = d = 64)
                    nc.tensor.matmul(ps[:],
                                     kT[dr:dr + D, dt, jt * P:(jt + 1) * P],
                                     qT[dr:dr + D, dt, :],
                                     start=True, stop=True)
                    e = e_pool.tile([P, Q], bf16, tag="e")
                    nc.scalar.activation(e[:], ps[:], Exp)
                    # attn_outT rows = [v | 1]^T @ E : row 64 = denom
                    nc.tensor.matmul(po[0:65, :],
                                     vA[:, jt, h * 65:(h + 1) * 65], e[:],
                                     start=(jt == 0), stop=(jt == NJT - 1))
                # denom row lives on psum partition 64; engines can't move
                # across partitions, so bounce via SBUF row 64 + DMA.
                den_t = den_pool.tile([P, Q], f32, tag="denrow")
                nc.scalar.copy(den_t[64:65, :], po[64:65, :])
                nc.sync.dma_start(den_dram[h:h + 1, :], den_t[64:65, :])
                nc.scalar.copy(aT[dr:dr + D, dt, :], po[0:64, :])

            # normalize by softmax denominator (head pair per partition
            # tile so both tensor_tensor inputs share base partition 0)
            nc.sync.dma_start(denoms[:], den_dram[:])
            nc.vector.reciprocal(recips[:], denoms[:])
            nc.sync.dma_start(rec_dram[:], recips[:])
            for g in range(ET):
                rbc = rbc_pool.tile([P, Q], f32, tag="rbc")
                nc.sync.dma_start(
                    rbc[0:D, :],
                    rec_dram[2 * g:2 * g + 1, :].to_broadcast((D, Q)))
                nc.sync.dma_start(
                    rbc[D:P, :],
                    rec_dram[2 * g + 1:2 * g + 2, :].to_broadcast((D, Q)))
                nc.vector.tensor_tensor(aT[:, g, :], aT[:, g, :], rbc[:],
                                        mybir.AluOpType.mult)

        # ---- phase 4: vocab projection ---------------------------------
        for vc in range(NVC):
            wo_t = wo_pool.tile([P, ET, VCH], bf16, tag="wo")
            nc.sync.dma_start(wo_t[:], wo3[:, :, vc * VCH:(vc + 1) * VCH])
            bo_t = bo_pool.tile([P, VCH], bf16, tag="bo")
            nc.sync.dma_start(
                bo_t[:],
                bo[vc * VCH:(vc + 1) * VCH]
                .rearrange("(o v) -> o v", o=1).to_broadcast((P, VCH)))
            for tc4 in range(Q // P):
                ps = psO.tile([P, VCH], f32)
                for et in range(ET):
                    nc.tensor.matmul(ps[:],
                                     aT[:, et, tc4 * P:(tc4 + 1) * P],
                                     wo_t[:, et, :],
                                     start=(et == 0), stop=(et == ET - 1))
                lt = lt_pool.tile([P, VCH], f32, tag="lt")
                nc.vector.tensor_tensor(lt[:], ps[:], bo_t[:],
                                        mybir.AluOpType.add)
                nc.sync.dma_start(
                    out[tc4 * P:(tc4 + 1) * P, vc * VCH:(vc + 1) * VCH],
                    lt[:])

    nc.compile()
    return nc


def get_nc():
    if "nc" not in _cache:
        _cache["nc"] = _build()
    return _cache["nc"]


def make_in_maps(x, Wq, bq, Wk, bk, Wv, bv, Wo, bo):
    import ml_dtypes

    def bf(a):
        return np.asarray(np.asarray(a, dtype=np.float32)
                          .astype(ml_dtypes.bfloat16))

    def f32a(a):
        return np.ascontiguousarray(np.asarray(a, dtype=np.float32))

    x = bf(x)
    Wq, Wk, Wv, Wo = bf(Wq), bf(Wk), bf(Wv), bf(Wo)
    bv, bo = bf(bv), bf(bo)
    bq, bk = f32a(bq), f32a(bk)
    in_maps = []
    for c in range(NCORES):
        b, qs = c // 4, (c % 4) * Q
        xbm = np.ascontiguousarray(np.roll(x[b], -qs, axis=0))
        in_maps.append({"xb": xbm, "wq": Wq, "wk": Wk, "wv": Wv, "wo": Wo,
                        "bq": bq, "bk": bk, "bv": bv, "bo": bo})
    return in_maps


def gather(results):
    out = np.empty((B, S, V), dtype=np.float32)
    for c in range(NCORES):
        b, qs = c // 4, (c % 4) * Q
        out[b, qs:qs + Q] = results[c]["out"]
    return out


def kernel(**inputs):
    from concourse.bass_utils import run_bass_kernel_spmd

    nc = get_nc()
    in_maps = make_in_maps(**inputs)
    res = run_bass_kernel_spmd(nc, in_maps, list(range(NCORES)), trace=False)
    return gather(res.results)


# revision 8
# speedup vs baseline: 1.4096x; 1.2451x over previous
"""Multi-head self-attention + vocab projection, 8-core TRN2 Bass kernel.

Problem: x[2,2048,1024] -> logits[2,2048,32000]
  q/k/v = x@W{q,k,v}+b, 16 heads x 64; attn = softmax(qk^T/8)v; out = attn@Wo+bo

Sharding: data-parallel over the 4096 token rows -> 8 cores x 512 query rows
(cores 0-3 batch 0, cores 4-7 batch 1). Each core receives its full batch
(2048 tokens) for K/V, ROLLED so that its 512 query rows are rows 0:512 —
softmax is permutation-invariant over the kv axis, so rolling is safe and
makes the SPMD program core-id independent. Wo is column-streamed in full on
every core; logits are written with no cross-core reduce.

x and all weights are converted to bf16 host-side (halves HBM traffic for
the dominant Wo stream; matmuls accumulate in f32 psum). xT comes from XBAR
DMA-transpose — no PE transposes. Attention is computed transposed
(scoresT[j,q] = kT^T qT) so exp(scoresT) feeds attn@V directly as lhsT and
the attention output lands as attn_outT[emb, tok] — exactly the lhsT layout
the vocab projection needs. The softmax denominator comes from an appended
ones-column on V; normalization is applied after attn@V via reciprocal +
DRAM-broadcast + elementwise multiply.

Projections and attention are INTERLEAVED per head-pair (the PE otherwise
micro-idles waiting on ACT exp between score/attn matmuls, which keeps the
HAM clock gate cold at 1.2 GHz — measured 300+ us of K=4/8 throttling in the
phase-separated version). Head-pair scores share one 2-bank psum tile so exp
runs once per kv-tile over [128, 1024]. All SBUF pools are top-level so Wo
prefetch DMAs can run during the attention phase.
"""

import numpy as np

B, S, E = 2, 2048, 1024
H, D = 16, 64
V = 32000
P = 128
ET = E // P          # 8 embedding tiles
TOK = S              # kv tokens per core
Q = 512              # query rows per core
NJT = TOK // P       # 16 kv tiles
VCH = 500            # vocab chunk (psum bank = 512 f32)
NVC = V // VCH       # 64
DVC = 256            # v-projection dout chunk (= 4 heads)
NCORES = 8

_cache = {}


def _build():
    from contextlib import ExitStack

    import concourse.tile as tile
    from concourse import bacc, mybir

    f32 = mybir.dt.float32
    bf16 = mybir.dt.bfloat16
    Id = mybir.ActivationFunctionType.Identity
    Exp = mybir.ActivationFunctionType.Exp

    nc = bacc.Bacc("TRN2", target_bir_lowering=False, debug=False,
                   num_devices=NCORES)

    xb = nc.dram_tensor("xb", [TOK, E], bf16, kind="ExternalInput").ap()
    wq = nc.dram_tensor("wq", [E, E], bf16, kind="ExternalInput").ap()
    wk = nc.dram_tensor("wk", [E, E], bf16, kind="ExternalInput").ap()
    wv = nc.dram_tensor("wv", [E, E], bf16, kind="ExternalInput").ap()
    wo = nc.dram_tensor("wo", [E, V], bf16, kind="ExternalInput").ap()
    bq = nc.dram_tensor("bq", [E], f32, kind="ExternalInput").ap()
    bk = nc.dram_tensor("bk", [E], f32, kind="ExternalInput").ap()
    bv = nc.dram_tensor("bv", [E], bf16, kind="ExternalInput").ap()
    bo = nc.dram_tensor("bo", [V], bf16, kind="ExternalInput").ap()
    out = nc.dram_tensor("out", [Q, V], f32, kind="ExternalOutput").ap()

    wq3 = wq.rearrange("(et p) d -> p et d", p=P)
    wk3 = wk.rearrange("(et p) d -> p et d", p=P)
    wv3 = wv.rearrange("(et p) d -> p et d", p=P)
    wo3 = wo.rearrange("(et p) v -> p et v", p=P)

    with tile.TileContext(nc) as tc, ExitStack() as ctx:
        # ---- pools (all top-level: scheduling is purely dep-driven) ----
        consts = ctx.enter_context(tc.tile_pool(name="consts", bufs=1))
        xT_pool = ctx.enter_context(tc.tile_pool(name="xT", bufs=1))
        kT_pool = ctx.enter_context(tc.tile_pool(name="kT", bufs=1))
        vA_pool = ctx.enter_context(tc.tile_pool(name="vA", bufs=1))
        qT_pool = ctx.enter_context(tc.tile_pool(name="qT", bufs=1))
        aT_pool = ctx.enter_context(tc.tile_pool(name="aT", bufs=1))
        dn_pool = ctx.enter_context(tc.tile_pool(name="dn", bufs=1))
        wqk_pool = ctx.enter_context(tc.tile_pool(name="wqk", bufs=2))
        wv_pool = ctx.enter_context(tc.tile_pool(name="wvp", bufs=2))
        e_pool = ctx.enter_context(tc.tile_pool(name="epool", bufs=4))
        den_pool = ctx.enter_context(tc.tile_pool(name="denrow", bufs=2))
        rbc_pool = ctx.enter_context(tc.tile_pool(name="rbc", bufs=2))
        wo_pool = ctx.enter_context(tc.tile_pool(name="wo", bufs=6))
        lt_pool = ctx.enter_context(tc.tile_pool(name="lt", bufs=4))
        bo_pool = ctx.enter_context(tc.tile_pool(name="bo", bufs=3))
        dram_pool = ctx.enter_context(
            tc.tile_pool(name="dramscratch", bufs=1, space="DRAM"))
        # PSUM: psS 2x2 banks + psA 2x1 + psM 2x1 = 8 banks exactly
        psS = ctx.enter_context(tc.tile_pool(name="psS", bufs=2,
                                             space="PSUM"))
        psA = ctx.enter_context(tc.tile_pool(name="psA", bufs=2,
                                             space="PSUM"))
        psM = ctx.enter_context(tc.tile_pool(name="psM", bufs=2,
                                             space="PSUM"))

        bq_sb = consts.tile([P, ET], f32)
        nc.sync.dma_start(bq_sb[:], bq.rearrange("(g p) -> p g", p=P))
        # fold the 1/sqrt(d) score scale into q: q' = (xWq + bq)/8
        nc.vector.tensor_scalar_mul(bq_sb[:], bq_sb[:], 0.125)
        bk_sb = consts.tile([P, ET], f32)
        nc.sync.dma_start(bk_sb[:], bk.rearrange("(g p) -> p g", p=P))
        bv_row = consts.tile([1, E], bf16)
        nc.sync.dma_start(bv_row[:], bv.rearrange("(o e) -> o e", o=1))
        ones_b = consts.tile([1, P], bf16)
        nc.vector.memset(ones_b[:], 1.0)

        xT = xT_pool.tile([P, ET, TOK], bf16)    # x^T, emb on partitions
        kT = kT_pool.tile([P, ET, TOK], bf16)    # k^T, d on partitions
        vA = vA_pool.tile([P, NJT, H * 65], bf16)  # v + ones col, per kv tile
        qT = qT_pool.tile([P, ET, Q], bf16)      # (q/8)^T
        aT = aT_pool.tile([P, ET, Q], bf16)      # attn_out^T
        denoms = dn_pool.tile([H, Q], f32)
        recips = dn_pool.tile([H, Q], f32)
        den_dram = dram_pool.tile([H, Q], f32)
        rec_dram = dram_pool.tile([H, Q], f32)

        # ones column of vA (head-local column 64)
        vA4 = vA[:].rearrange("p j (h c) -> p j h c", c=65)
        nc.vector.memset(vA4[:, :, :, 64:65], 1.0)

        # x^T via XBAR dma-transpose
        for et in range(ET):
            nc.sync.dma_start_transpose(xT[:, et, :],
                                        xb[:, et * P:(et + 1) * P])

        # ---- interleaved projections + attention, one head pair per g --
        for g in range(ET):
            # project kT[:, g, :] (dout tile g = heads 2g, 2g+1)
            wk_t = wqk_pool.tile([P, ET, P], bf16, tag="wqk")
            nc.sync.dma_start(wk_t[:], wk3[:, :, g * P:(g + 1) * P])
            for tcc in range(TOK // 512):
                ps = psM.tile([P, 512], f32, tag="m")
                for et in range(ET):
                    nc.tensor.matmul(ps[:], wk_t[:, et, :],
                                     xT[:, et, tcc * 512:(tcc + 1) * 512],
                                     start=(et == 0), stop=(et == ET - 1))
                nc.scalar.activation(kT[:, g, tcc * 512:(tcc + 1) * 512],
                                     ps[:], Id, bias=bk_sb[:, g:g + 1])
            # project qT[:, g, :]
            wq_t = wqk_pool.tile([P, ET, P], bf16, tag="wqk")
            nc.sync.dma_start(wq_t[:], wq3[:, :, g * P:(g + 1) * P])
            ps = psM.tile([P, 512], f32, tag="m")
            for et in range(ET):
                nc.tensor.matmul(ps[:], wq_t[:, et, :], xT[:, et, 0:Q],
                                 start=(et == 0), stop=(et == ET - 1))
            nc.scalar.activation(qT[:, g, :], ps[:], Id,
                                 bias=bq_sb[:, g:g + 1], scale=0.125)

            # project v chunk dvc=g//2 (heads 4*(g//2) .. +3) on even g
            if g % 2 == 0:
                dvc = g // 2
                wv_t = wv_pool.tile([P, ET, DVC], bf16, tag="wv")
                nc.sync.dma_start(wv_t[:],
                                  wv3[:, :, dvc * DVC:(dvc + 1) * DVC])
                for tt in range(NJT):
                    ps = psM.tile([P, 512], f32, tag="m")
                    psv = ps[:, 0:DVC]
                    for et in range(ET):
                        nc.tensor.matmul(psv,
                                         xT[:, et, tt * P:(tt + 1) * P],
                                         wv_t[:, et, :],
                                         start=(et == 0), stop=False)
                    nc.tensor.matmul(
                        psv, ones_b[0:1, :],
                        bv_row[0:1, dvc * DVC:(dvc + 1) * DVC],
                        start=False, stop=True)
                    h0 = dvc * (DVC // D)
                    dst = vA4[:, tt, h0:h0 + DVC // D, 0:64]
                    nc.vector.tensor_copy(
                        dst, psv.rearrange("p (h c) -> p h c", c=D))

            # attention for heads 2g (rows 0:64) and 2g+1 (rows 64:128)
            h0, h1 = 2 * g, 2 * g + 1
            po0 = psA.tile([P, Q], f32, tag="a")
            po1 = psA.tile([P, Q], f32, tag="a")
            for jt in range(NJT):
                ps = psS.tile([P, 2 * Q], f32, tag="s")
                nc.tensor.matmul(ps[:, 0:Q],
                                 kT[0:D, g, jt * P:(jt + 1) * P],
                                 qT[0:D, g, :], start=True, stop=True)
                nc.tensor.matmul(ps[:, Q:2 * Q],
                                 kT[D:P, g, jt * P:(jt + 1) * P],
                                 qT[D:P, g, :], start=True, stop=True)
                e = e_pool.tile([P, 2 * Q], bf16, tag="e")
                nc.scalar.activation(e[:], ps[:], Exp)
                nc.tensor.matmul(po0[0:65, :],
                                 vA[:, jt, h0 * 65:h0 * 65 + 65],
                                 e[:, 0:Q],
                                 start=(jt == 0), stop=(jt == NJT - 1))
                nc.tensor.matmul(po1[0:65, :],
                                 vA[:, jt, h1 * 65:h1 * 65 + 65],
                                 e[:, Q:2 * Q],
                                 start=(jt == 0), stop=(jt == NJT - 1))
            # denom rows live on psum partition 64; engines can't move
            # across partitions, so bounce via SBUF row 64 + DMA.
            for po, hh, dr in ((po0, h0, 0), (po1, h1, D)):
                den_t = den_pool.tile([P, Q], f32, tag="denrow")
                nc.scalar.copy(den_t[64:65, :], po[64:65, :])
                nc.sync.dma_start(den_dram[hh:hh + 1, :], den_t[64:65, :])
                nc.scalar.copy(aT[dr:dr + D, g, :], po[0:64, :])

        # ---- normalize by softmax denominator (head pair per partition
        # tile so both tensor_tensor inputs share base partition 0) ------
        nc.sync.dma_start(denoms[:], den_dram[:])
        nc.vector.reciprocal(recips[:], denoms[:])
        nc.sync.dma_start(rec_dram[:], recips[:])
        for g in range(ET):
            rbc = rbc_pool.tile([P, Q], f32, tag="rbc")
            nc.sync.dma_start(
                rbc[0:D, :],
                rec_dram[2 * g:2 * g + 1, :].to_broadcast((D, Q)))
            nc.sync.dma_start(
                rbc[D:P, :],
                rec_dram[2 * g + 1:2 * g + 2, :].to_broadcast((D, Q)))
            nc.vector.tensor_tensor(aT[:, g, :], aT[:, g, :], rbc[:],
                                    mybir.AluOpType.mult)

        # ---- vocab projection ------------------------------------------
        for vc in range(NVC):
            wo_t = wo_pool.tile([P, ET, VCH], bf16, tag="wo")
            nc.sync.dma_start(wo_t[:], wo3[:, :, vc * VCH:(vc + 1) * VCH])
            bo_t = bo_pool.tile([P, VCH], bf16, tag="bo")
            nc.sync.dma_start(
                bo_t[:],
                bo[vc * VCH:(vc + 1) * VCH]
                .rearrange("(o v) -> o v", o=1).to_broadcast((P, VCH)))
            for tc4 in range(Q // P):
                ps = psM.tile([P, 512], f32, tag="m")
                pso = ps[:, 0:VCH]
                for et in range(ET):
                    nc.tensor.matmul(pso,
                                     aT[:, et, tc4 * P:(tc4 + 1) * P],
                                     wo_t[:, et, :],
                                     start=(et == 0), stop=(et == ET - 1))
                lt = lt_pool.tile([P, VCH], f32, tag="lt")
                nc.vector.tensor_tensor(lt[:], pso, bo_t[:],
                                        mybir.AluOpType.add)
                nc.sync.dma_start(
                    out[tc4 * P:(tc4 + 1) * P, vc * VCH:(vc + 1) * VCH],
                    lt[:])

    nc.compile()
    return nc


def get_nc():
    if "nc" not in _cache:
        _cache["nc"] = _build()
    return _cache["nc"]


def make_in_maps(x, Wq, bq, Wk, bk, Wv, bv, Wo, bo):
    import ml_dtypes

    def bf(a):
        return np.asarray(np.asarray(a, dtype=np.float32)
                          .astype(ml_dtypes.bfloat16))

    def f32a(a):
        return np.ascontiguousarray(np.asarray(a, dtype=np.float32))

    x = bf(x)
    Wq, Wk, Wv, Wo = bf(Wq), bf(Wk), bf(Wv), bf(Wo)
    bv, bo = bf(bv), bf(bo)
    bq, bk = f32a(bq), f32a(bk)
    in_maps = []
    for c in range(NCORES):
        b, qs = c // 4, (c % 4) * Q
        xbm = np.ascontiguousarray(np.roll(x[b], -qs, axis=0))
        in_maps.append({"xb": xbm, "wq": Wq, "wk": Wk, "wv": Wv, "wo": Wo,
                        "bq": bq, "bk": bk, "bv": bv, "bo": bo})
    return in_maps


def gather(results):
    out = np.empty((B, S, V), dtype=np.float32)
    for c in range(NCORES):
        b, qs = c // 4, (c % 4) * Q
        out[b, qs:qs + Q] = results[c]["out"]
    return out


def kernel(**inputs):
    from concourse.bass_utils import run_bass_kernel_spmd

    nc = get_nc()
    in_maps = make_in_maps(**inputs)
    res = run_bass_kernel_spmd(nc, in_maps, list(range(NCORES)), trace=False)
    return gather(res.results)


# revision 14
# speedup vs baseline: 1.4158x; 1.0044x over previous
"""Multi-head self-attention + vocab projection, 8-core TRN2 Bass kernel.

Problem: x[2,2048,1024] -> logits[2,2048,32000]
  q/k/v = x@W{q,k,v}+b, 16 heads x 64; attn = softmax(qk^T/8)v; out = attn@Wo+bo

Sharding: data-parallel over the 4096 token rows -> 8 cores x 512 query rows
(cores 0-3 batch 0, cores 4-7 batch 1). Each core receives its full batch
(2048 tokens) for K/V, ROLLED so that its 512 query rows are rows 0:512 —
softmax is permutation-invariant over the kv axis, so rolling is safe and
makes the SPMD program core-id independent. Wo is column-streamed in full on
every core; logits are written with no cross-core reduce.

x and all weights are converted to bf16 host-side (halves HBM traffic for
the dominant Wo stream; matmuls accumulate in f32 psum). xT comes from XBAR
DMA-transpose — no PE transposes. Attention is computed transposed
(scoresT[j,q] = kT^T qT) so exp(scoresT) feeds attn@V directly as lhsT and
the attention output lands as attn_outT[emb, tok] — exactly the lhsT layout
the vocab projection needs. The softmax denominator comes from an appended
ones-column on V; normalization is applied after attn@V via reciprocal +
DRAM-broadcast + elementwise multiply.

Projections and attention are INTERLEAVED per head-pair (the PE otherwise
micro-idles waiting on ACT exp between score/attn matmuls, which keeps the
HAM clock gate cold at 1.2 GHz — measured 300+ us of K=4/8 throttling in the
phase-separated version). Head-pair scores share one 2-bank psum tile so exp
runs once per kv-tile over [128, 1024]. All SBUF pools are top-level so Wo
prefetch DMAs can run during the attention phase.
"""

import numpy as np

B, S, E = 2, 2048, 1024
H, D = 16, 64
V = 32000
P = 128
ET = E // P          # 8 embedding tiles
TOK = S              # kv tokens per core
Q = 512              # query rows per core
NJT = TOK // P       # 16 kv tiles
VCH = 500            # vocab chunk (psum bank = 512 f32)
NVC = V // VCH       # 64
DVC = 256            # v-projection dout chunk (= 4 heads)
NCORES = 8

_cache = {}


def _build():
    from contextlib import ExitStack

    import concourse.tile as tile
    from concourse import bacc, mybir

    f32 = mybir.dt.float32
    bf16 = mybir.dt.bfloat16
    Id = mybir.ActivationFunctionType.Identity
    Exp = mybir.ActivationFunctionType.Exp

    nc = bacc.Bacc("TRN2", target_bir_lowering=False, debug=False,
                   num_devices=NCORES)

    xb = nc.dram_tensor("xb", [TOK, E], bf16, kind="ExternalInput").ap()
    wq = nc.dram_tensor("wq", [E, E], bf16, kind="ExternalInput").ap()
    wk = nc.dram_tensor("wk", [E, E], bf16, kind="ExternalInput").ap()
    wv = nc.dram_tensor("wv", [E, E], bf16, kind="ExternalInput").ap()
    wo = nc.dram_tensor("wo", [E, V], bf16, kind="ExternalInput").ap()
    bq = nc.dram_tensor("bq", [E], f32, kind="ExternalInput").ap()
    bk = nc.dram_tensor("bk", [E], f32, kind="ExternalInput").ap()
    bv = nc.dram_tensor("bv", [E], bf16, kind="ExternalInput").ap()
    bo = nc.dram_tensor("bo", [V], bf16, kind="ExternalInput").ap()
    out = nc.dram_tensor("out", [Q, V], f32, kind="ExternalOutput").ap()

    wq3 = wq.rearrange("(et p) d -> p et d", p=P)
    wk3 = wk.rearrange("(et p) d -> p et d", p=P)
    wv3 = wv.rearrange("(et p) d -> p et d", p=P)
    wo3 = wo.rearrange("(et p) v -> p et v", p=P)

    with tile.TileContext(nc) as tc, ExitStack() as ctx:
        # ---- pools (all top-level: scheduling is purely dep-driven) ----
        consts = ctx.enter_context(tc.tile_pool(name="consts", bufs=1))
        xT_pool = ctx.enter_context(tc.tile_pool(name="xT", bufs=1))
        kT_pool = ctx.enter_context(tc.tile_pool(name="kT", bufs=1))
        vA_pool = ctx.enter_context(tc.tile_pool(name="vA", bufs=1))
        qT_pool = ctx.enter_context(tc.tile_pool(name="qT", bufs=1))
        aT_pool = ctx.enter_context(tc.tile_pool(name="aT", bufs=1))
        dn_pool = ctx.enter_context(tc.tile_pool(name="dn", bufs=1))
        wqk_pool = ctx.enter_context(tc.tile_pool(name="wqk", bufs=2))
        wv_pool = ctx.enter_context(tc.tile_pool(name="wvp", bufs=2))
        e_pool = ctx.enter_context(tc.tile_pool(name="epool", bufs=3))
        den_pool = ctx.enter_context(tc.tile_pool(name="denrow", bufs=2))
        den2_pool = ctx.enter_context(tc.tile_pool(name="den2", bufs=1))
        rbc_pool = ctx.enter_context(tc.tile_pool(name="rbc", bufs=2))
        wo_pool = ctx.enter_context(tc.tile_pool(name="wo", bufs=5))
        lt_pool = ctx.enter_context(tc.tile_pool(name="lt", bufs=4))
        bo_pool = ctx.enter_context(tc.tile_pool(name="bo", bufs=3))
        dram_pool = ctx.enter_context(
            tc.tile_pool(name="dramscratch", bufs=1, space="DRAM"))
        # PSUM: shared 2-bank-slot pool (3 bufs) + attention accums (2x1
        # bank) = 8 banks exactly
        psP = ctx.enter_context(tc.tile_pool(name="ps", bufs=3,
                                             space="PSUM"))
        psA = ctx.enter_context(tc.tile_pool(name="psA", bufs=2,
                                             space="PSUM"))

        bq_sb = consts.tile([P, ET], f32)
        nc.sync.dma_start(bq_sb[:], bq.rearrange("(g p) -> p g", p=P))
        # fold the 1/sqrt(d) score scale into q: q' = (xWq + bq)/8
        nc.vector.tensor_scalar_mul(bq_sb[:], bq_sb[:], 0.125)
        bk_sb = consts.tile([P, ET], f32)
        nc.sync.dma_start(bk_sb[:], bk.rearrange("(g p) -> p g", p=P))
        bv_bc = consts.tile([P, E], bf16)
        nc.sync.dma_start(
            bv_bc[:],
            bv.rearrange("(o e) -> o e", o=1).to_broadcast((P, E)))

        xT = xT_pool.tile([P, ET, TOK], bf16)    # x^T, emb on partitions
        kT = kT_pool.tile([P, ET, TOK], bf16)    # k^T, d on partitions
        vA = vA_pool.tile([P, NJT, H * 65], bf16)  # v + ones col, per kv tile
        qT = qT_pool.tile([P, ET, Q], bf16)      # (q/8)^T
        aT = aT_pool.tile([P, ET, Q], bf16)      # attn_out^T
        den_dram = dram_pool.tile([H, Q], f32)
        rec_dram = dram_pool.tile([H, Q], f32)

        # ones column of vA (head-local column 64)
        vA4 = vA[:].rearrange("p j (h c) -> p j h c", c=65)
        nc.vector.memset(vA4[:, :, :, 64:65], 1.0)

        # x^T via XBAR dma-transpose, split across both HWDGE queues
        for et in range(ET):
            eng = nc.sync if et % 2 == 0 else nc.scalar
            eng.dma_start_transpose(xT[:, et, :],
                                    xb[:, et * P:(et + 1) * P])

        # ---- interleaved projections + attention, one head pair per g --
        for g in range(ET):
            # project kT[:, g, :] (dout tile g = heads 2g, 2g+1)
            wk_t = wqk_pool.tile([P, ET, P], bf16, tag="wqk")
            nc.sync.dma_start(wk_t[:], wk3[:, :, g * P:(g + 1) * P])
            for tcc in range(TOK // 512):
                ps = psP.tile([P, 1024], f32, tag="ps")
                for et in range(ET):
                    nc.tensor.matmul(ps[:, 0:512], wk_t[:, et, :],
                                     xT[:, et, tcc * 512:(tcc + 1) * 512],
                                     start=(et == 0), stop=(et == ET - 1))
                nc.vector.tensor_scalar_add(
                    kT[:, g, tcc * 512:(tcc + 1) * 512], ps[:, 0:512],
                    bk_sb[:, g:g + 1])
            # project qT[:, g, :] (score scale 1/8 folded in)
            wq_t = wqk_pool.tile([P, ET, P], bf16, tag="wqk")
            nc.sync.dma_start(wq_t[:], wq3[:, :, g * P:(g + 1) * P])
            ps = psP.tile([P, 1024], f32, tag="ps")
            for et in range(ET):
                nc.tensor.matmul(ps[:, 0:512], wq_t[:, et, :], xT[:, et, 0:Q],
                                 start=(et == 0), stop=(et == ET - 1))
            nc.vector.tensor_scalar(qT[:, g, :], ps[:, 0:512], 0.125,
                                    bq_sb[:, g:g + 1],
                                    mybir.AluOpType.mult,
                                    mybir.AluOpType.add)

            # project v chunk dvc=g//2 (heads 4*(g//2) .. +3) on even g
            if g % 2 == 0:
                dvc = g // 2
                wv_t = wv_pool.tile([P, ET, DVC], bf16, tag="wv")
                nc.sync.dma_start(wv_t[:],
                                  wv3[:, :, dvc * DVC:(dvc + 1) * DVC])
                for tt in range(NJT):
                    ps = psP.tile([P, 1024], f32, tag="ps")
                    psv = ps[:, 0:DVC]
                    for et in range(ET):
                        nc.tensor.matmul(psv,
                                         xT[:, et, tt * P:(tt + 1) * P],
                                         wv_t[:, et, :],
                                         start=(et == 0),
                                         stop=(et == ET - 1))
                    h0 = dvc * (DVC // D)
                    dst = vA4[:, tt, h0:h0 + DVC // D, 0:64]
                    bvs = bv_bc[:, dvc * DVC:(dvc + 1) * DVC]
                    nc.vector.tensor_tensor(
                        dst, psv.rearrange("p (h c) -> p h c", c=D),
                        bvs.rearrange("p (h c) -> p h c", c=D),
                        mybir.AluOpType.add)

            # attention for heads 2g (rows 0:64) and 2g+1 (rows 64:128);
            # the two K=64 score matmuls pack into array row-halves via
            # tile_position and run concurrently.
            h0, h1 = 2 * g, 2 * g + 1
            po0 = psA.tile([P, Q], f32, tag="a")
            po1 = psA.tile([P, Q], f32, tag="a")
            for jt in range(NJT):
                ps = psP.tile([P, 2 * Q], f32, tag="ps")
                nc.tensor.matmul(ps[:, 0:Q],
                                 kT[0:D, g, jt * P:(jt + 1) * P],
                                 qT[0:D, g, :], start=True, stop=True,
                                 tile_position=(0, 0))
                nc.tensor.matmul(ps[:, Q:2 * Q],
                                 kT[D:P, g, jt * P:(jt + 1) * P],
                                 qT[D:P, g, :], start=True, stop=True,
                                 tile_position=(64, 0))
                e = e_pool.tile([P, 2 * Q], bf16, tag="e")
                nc.scalar.activation(e[:], ps[:], Exp)
                nc.tensor.matmul(po0[0:65, :],
                                 vA[:, jt, h0 * 65:h0 * 65 + 65],
                                 e[:, 0:Q],
                                 start=(jt == 0), stop=(jt == NJT - 1))
                nc.tensor.matmul(po1[0:65, :],
                                 vA[:, jt, h1 * 65:h1 * 65 + 65],
                                 e[:, Q:2 * Q],
                                 start=(jt == 0), stop=(jt == NJT - 1))
            # denom rows live on psum partition 64; engines can't move
            # across partitions, so bounce via SBUF row 64 + DMA.
            for po, hh, dr in ((po0, h0, 0), (po1, h1, D)):
                den_t = den_pool.tile([P, Q], f32, tag="denrow")
                nc.scalar.copy(den_t[64:65, :], po[64:65, :])
                nc.sync.dma_start(den_dram[hh:hh + 1, :], den_t[64:65, :])
                nc.scalar.copy(aT[dr:dr + D, g, :], po[0:64, :])

            # normalize this pair by its softmax denominators (per-pair so
            # the chain overlaps later pairs' attention instead of
            # serializing at the end)
            den2 = den2_pool.tile([2, Q], f32, tag="den2")
            nc.sync.dma_start(den2[:], den_dram[h0:h1 + 1, :])
            rec2 = den2_pool.tile([2, Q], f32, tag="rec2")
            nc.vector.reciprocal(rec2[:], den2[:])
            nc.sync.dma_start(rec_dram[h0:h1 + 1, :], rec2[:])
            rbc = rbc_pool.tile([P, Q], f32, tag="rbc")
            nc.sync.dma_start(
                rbc[0:D, :], rec_dram[h0:h0 + 1, :].to_broadcast((D, Q)))
            nc.sync.dma_start(
                rbc[D:P, :], rec_dram[h1:h1 + 1, :].to_broadcast((D, Q)))
            nc.vector.tensor_tensor(aT[:, g, :], aT[:, g, :], rbc[:],
                                    mybir.AluOpType.mult)

        # ---- vocab projection ------------------------------------------
        for vc in range(NVC):
            wo_t = wo_pool.tile([P, ET, VCH], bf16, tag="wo")
            nc.sync.dma_start(wo_t[:], wo3[:, :, vc * VCH:(vc + 1) * VCH])
            bo_t = bo_pool.tile([P, VCH], bf16, tag="bo")
            nc.sync.dma_start(
                bo_t[:],
                bo[vc * VCH:(vc + 1) * VCH]
                .rearrange("(o v) -> o v", o=1).to_broadcast((P, VCH)))
            for tc4 in range(Q // P):
                ps = psP.tile([P, 1024], f32, tag="ps")
                pso = ps[:, 0:VCH]
                for et in range(ET):
                    nc.tensor.matmul(pso,
                                     aT[:, et, tc4 * P:(tc4 + 1) * P],
                                     wo_t[:, et, :],
                                     start=(et == 0), stop=(et == ET - 1))
                lt = lt_pool.tile([P, VCH], f32, tag="lt")
                nc.vector.tensor_tensor(lt[:], pso, bo_t[:],
                                        mybir.AluOpType.add)
                nc.scalar.dma_start(
                    out[tc4 * P:(tc4 + 1) * P, vc * VCH:(vc + 1) * VCH],
                    lt[:])

    nc.compile()
    return nc


def get_nc():
    if "nc" not in _cache:
        _cache["nc"] = _build()
    return _cache["nc"]


def make_in_maps(x, Wq, bq, Wk, bk, Wv, bv, Wo, bo):
    import ml_dtypes

    def bf(a):
        return np.asarray(np.asarray(a, dtype=np.float32)
                          .astype(ml_dtypes.bfloat16))

    def f32a(a):
        return np.ascontiguousarray(np.asarray(a, dtype=np.float32))

    x = bf(x)
    Wq, Wk, Wv, Wo = bf(Wq), bf(Wk), bf(Wv), bf(Wo)
    bv, bo = bf(bv), bf(bo)
    bq, bk = f32a(bq), f32a(bk)
    in_maps = []
    for c in range(NCORES):
        b, qs = c // 4, (c % 4) * Q
        xbm = np.ascontiguousarray(np.roll(x[b], -qs, axis=0))
        in_maps.append({"xb": xbm, "wq": Wq, "wk": Wk, "wv": Wv, "wo": Wo,
                        "bq": bq, "bk": bk, "bv": bv, "bo": bo})
    return in_maps


def gather(results):
    out = np.empty((B, S, V), dtype=np.float32)
    for c in range(NCORES):
        b, qs = c // 4, (c % 4) * Q
        out[b, qs:qs + Q] = results[c]["out"]
    return out


def kernel(**inputs):
    from concourse.bass_utils import run_bass_kernel_spmd

    nc = get_nc()
    in_maps = make_in_maps(**inputs)
    res = run_bass_kernel_spmd(nc, in_maps, list(range(NCORES)), trace=False)
    return gather(res.results)


# revision 19
# speedup vs baseline: 1.5470x; 1.0927x over previous
"""Multi-head self-attention + vocab projection, 8-core TRN2 Bass kernel.

Problem: x[2,2048,1024] -> logits[2,2048,32000]
  q/k/v = x@W{q,k,v}+b, 16 heads x 64; attn = softmax(qk^T/8)v; out = attn@Wo+bo

Sharding: data-parallel over the 4096 token rows -> 8 cores x 512 query rows
(cores 0-3 batch 0, cores 4-7 batch 1). Each core receives its full batch
(2048 tokens) for K/V, ROLLED so that its 512 query rows are rows 0:512 —
softmax is permutation-invariant over the kv axis, so rolling is safe and
makes the SPMD program core-id independent. Wo is column-streamed in full on
every core; logits are written with no cross-core reduce.

x and all weights are converted to bf16 host-side (halves HBM traffic for
the dominant Wo stream; matmuls accumulate in f32 psum). xT comes from XBAR
DMA-transpose — no PE transposes. Attention is computed transposed
(scoresT[j,q] = kT^T qT) so exp(scoresT) feeds attn@V directly as lhsT and
the attention output lands as attn_outT[emb, tok] — exactly the lhsT layout
the vocab projection needs. The softmax denominator comes from an appended
ones-column on V; normalization is applied after attn@V via reciprocal +
DRAM-broadcast + elementwise multiply.

Projections and attention are INTERLEAVED per head-pair (the PE otherwise
micro-idles waiting on ACT exp between score/attn matmuls, which keeps the
HAM clock gate cold at 1.2 GHz — measured 300+ us of K=4/8 throttling in the
phase-separated version). Head-pair scores share one 2-bank psum tile so exp
runs once per kv-tile over [128, 1024]. All SBUF pools are top-level so Wo
prefetch DMAs can run during the attention phase.
"""

import numpy as np

B, S, E = 2, 2048, 1024
H, D = 16, 64
V = 32000
P = 128
ET = E // P          # 8 embedding tiles
TOK = S              # kv tokens per core
Q = 512              # query rows per core
NJT = TOK // P       # 16 kv tiles
VCH = 500            # vocab chunk (psum bank = 512 f32)
NVC = V // VCH       # 64
DVC = 256            # v-projection dout chunk (= 4 heads)
NCORES = 8

_cache = {}


def _build():
    from contextlib import ExitStack

    import concourse.tile as tile
    from concourse import bacc, mybir

    f32 = mybir.dt.float32
    bf16 = mybir.dt.bfloat16
    Id = mybir.ActivationFunctionType.Identity
    Exp = mybir.ActivationFunctionType.Exp

    nc = bacc.Bacc("TRN2", target_bir_lowering=False, debug=False,
                   num_devices=NCORES)

    xb = nc.dram_tensor("xb", [TOK, E], bf16, kind="ExternalInput").ap()
    wq = nc.dram_tensor("wq", [E, E], bf16, kind="ExternalInput").ap()
    wk = nc.dram_tensor("wk", [E, E], bf16, kind="ExternalInput").ap()
    wv = nc.dram_tensor("wv", [E, E], bf16, kind="ExternalInput").ap()
    wo = nc.dram_tensor("wo", [E, V], bf16, kind="ExternalInput").ap()
    bq = nc.dram_tensor("bq", [E], f32, kind="ExternalInput").ap()
    bk = nc.dram_tensor("bk", [E], f32, kind="ExternalInput").ap()
    bv = nc.dram_tensor("bv", [E], bf16, kind="ExternalInput").ap()
    bo = nc.dram_tensor("bo", [V], bf16, kind="ExternalInput").ap()
    out = nc.dram_tensor("out", [Q, V], bf16, kind="ExternalOutput").ap()

    wq3 = wq.rearrange("(et p) d -> p et d", p=P)
    wk3 = wk.rearrange("(et p) d -> p et d", p=P)
    wv3 = wv.rearrange("(et p) d -> p et d", p=P)
    wo3 = wo.rearrange("(et p) v -> p et v", p=P)

    with tile.TileContext(nc) as tc, ExitStack() as ctx:
        # ---- pools (all top-level: scheduling is purely dep-driven) ----
        consts = ctx.enter_context(tc.tile_pool(name="consts", bufs=1))
        xT_pool = ctx.enter_context(tc.tile_pool(name="xT", bufs=1))
        kT_pool = ctx.enter_context(tc.tile_pool(name="kT", bufs=1))
        vA_pool = ctx.enter_context(tc.tile_pool(name="vA", bufs=1))
        qT_pool = ctx.enter_context(tc.tile_pool(name="qT", bufs=1))
        aT_pool = ctx.enter_context(tc.tile_pool(name="aT", bufs=1))
        dn_pool = ctx.enter_context(tc.tile_pool(name="dn", bufs=1))
        wqk_pool = ctx.enter_context(tc.tile_pool(name="wqk", bufs=2))
        wv_pool = ctx.enter_context(tc.tile_pool(name="wvp", bufs=2))
        e_pool = ctx.enter_context(tc.tile_pool(name="epool", bufs=3))
        den_pool = ctx.enter_context(tc.tile_pool(name="denrow", bufs=2))
        den2_pool = ctx.enter_context(tc.tile_pool(name="den2", bufs=1))
        rbc_pool = ctx.enter_context(tc.tile_pool(name="rbc", bufs=2))
        wo_pool = ctx.enter_context(tc.tile_pool(name="wo", bufs=5))
        lt_pool = ctx.enter_context(tc.tile_pool(name="lt", bufs=4))
        bo_pool = ctx.enter_context(tc.tile_pool(name="bo", bufs=3))
        dram_pool = ctx.enter_context(
            tc.tile_pool(name="dramscratch", bufs=1, space="DRAM"))
        # PSUM: shared 2-bank-slot pool (3 bufs) + attention accums (2x1
        # bank) = 8 banks exactly
        psP = ctx.enter_context(tc.tile_pool(name="ps", bufs=3,
                                             space="PSUM"))
        psA = ctx.enter_context(tc.tile_pool(name="psA", bufs=2,
                                             space="PSUM"))

        bq_sb = consts.tile([P, ET], f32)
        nc.sync.dma_start(bq_sb[:], bq.rearrange("(g p) -> p g", p=P))
        # fold the 1/sqrt(d) score scale into q: q' = (xWq + bq)/8
        nc.vector.tensor_scalar_mul(bq_sb[:], bq_sb[:], 0.125)
        bk_sb = consts.tile([P, ET], f32)
        nc.sync.dma_start(bk_sb[:], bk.rearrange("(g p) -> p g", p=P))
        bv_bc = consts.tile([P, E], bf16)
        nc.sync.dma_start(
            bv_bc[:],
            bv.rearrange("(o e) -> o e", o=1).to_broadcast((P, E)))

        xT = xT_pool.tile([P, ET, TOK], bf16)    # x^T, emb on partitions
        kT = kT_pool.tile([P, ET, TOK], bf16)    # k^T, d on partitions
        vA = vA_pool.tile([P, NJT, H * 65], bf16)  # v + ones col, per kv tile
        qT = qT_pool.tile([P, ET, Q], bf16)      # (q/8)^T
        aT = aT_pool.tile([P, ET, Q], bf16)      # attn_out^T
        den_dram = dram_pool.tile([H, Q], f32)
        rec_dram = dram_pool.tile([H, Q], f32)

        # ones column of vA (head-local column 64)
        vA4 = vA[:].rearrange("p j (h c) -> p j h c", c=65)
        nc.vector.memset(vA4[:, :, :, 64:65], 1.0)

        # prefetch the first weight tiles BEFORE the transposes so the
        # first projection matmuls aren't queued behind them
        wk0 = wqk_pool.tile([P, ET, P], bf16, tag="wqk")
        nc.sync.dma_start(wk0[:], wk3[:, :, 0:P])
        wv0 = wv_pool.tile([P, ET, DVC], bf16, tag="wv")
        nc.scalar.dma_start(wv0[:], wv3[:, :, 0:DVC])

        # x^T via XBAR dma-transpose, split across both HWDGE queues
        for et in range(ET):
            eng = nc.sync if et % 2 == 0 else nc.scalar
            eng.dma_start_transpose(xT[:, et, :],
                                    xb[:, et * P:(et + 1) * P])

        # ---- interleaved projections + attention, one head pair per g --
        for g in range(ET):
            # project kT[:, g, :] (dout tile g = heads 2g, 2g+1)
            if g == 0:
                wk_t = wk0
            else:
                wk_t = wqk_pool.tile([P, ET, P], bf16, tag="wqk")
                nc.sync.dma_start(wk_t[:], wk3[:, :, g * P:(g + 1) * P])
            for tcc in range(TOK // 512):
                ps = psP.tile([P, 1024], f32, tag="ps")
                for et in range(ET):
                    nc.tensor.matmul(ps[:, 0:512], wk_t[:, et, :],
                                     xT[:, et, tcc * 512:(tcc + 1) * 512],
                                     start=(et == 0), stop=(et == ET - 1))
                nc.vector.tensor_scalar_add(
                    kT[:, g, tcc * 512:(tcc + 1) * 512], ps[:, 0:512],
                    bk_sb[:, g:g + 1])
            # project qT[:, g, :] (score scale 1/8 folded in)
            wq_t = wqk_pool.tile([P, ET, P], bf16, tag="wqk")
            nc.sync.dma_start(wq_t[:], wq3[:, :, g * P:(g + 1) * P])
            ps = psP.tile([P, 1024], f32, tag="ps")
            for et in range(ET):
                nc.tensor.matmul(ps[:, 0:512], wq_t[:, et, :], xT[:, et, 0:Q],
                                 start=(et == 0), stop=(et == ET - 1))
            nc.vector.tensor_scalar(qT[:, g, :], ps[:, 0:512], 0.125,
                                    bq_sb[:, g:g + 1],
                                    mybir.AluOpType.mult,
                                    mybir.AluOpType.add)

            # project v chunk dvc=g//2 (heads 4*(g//2) .. +3) on even g
            if g % 2 == 0:
                dvc = g // 2
                if dvc == 0:
                    wv_t = wv0
                else:
                    wv_t = wv_pool.tile([P, ET, DVC], bf16, tag="wv")
                    nc.sync.dma_start(wv_t[:],
                                      wv3[:, :, dvc * DVC:(dvc + 1) * DVC])
                for tt in range(NJT):
                    ps = psP.tile([P, 1024], f32, tag="ps")
                    psv = ps[:, 0:DVC]
                    for et in range(ET):
                        nc.tensor.matmul(psv,
                                         xT[:, et, tt * P:(tt + 1) * P],
                                         wv_t[:, et, :],
                                         start=(et == 0),
                                         stop=(et == ET - 1))
                    h0 = dvc * (DVC // D)
                    dst = vA4[:, tt, h0:h0 + DVC // D, 0:64]
                    bvs = bv_bc[:, dvc * DVC:(dvc + 1) * DVC]
                    nc.vector.tensor_tensor(
                        dst, psv.rearrange("p (h c) -> p h c", c=D),
                        bvs.rearrange("p (h c) -> p h c", c=D),
                        mybir.AluOpType.add)

            # attention for heads 2g (rows 0:64) and 2g+1 (rows 64:128);
            # the two K=64 score matmuls pack into array row-halves via
            # tile_position and run concurrently.
            h0, h1 = 2 * g, 2 * g + 1
            po0 = psA.tile([P, Q], f32, tag="a")
            po1 = psA.tile([P, Q], f32, tag="a")
            for jt in range(NJT):
                ps = psP.tile([P, 2 * Q], f32, tag="ps")
                nc.tensor.matmul(ps[:, 0:Q],
                                 kT[0:D, g, jt * P:(jt + 1) * P],
                                 qT[0:D, g, :], start=True, stop=True,
                                 tile_position=(0, 0))
                nc.tensor.matmul(ps[:, Q:2 * Q],
                                 kT[D:P, g, jt * P:(jt + 1) * P],
                                 qT[D:P, g, :], start=True, stop=True,
                                 tile_position=(64, 0))
                e = e_pool.tile([P, 2 * Q], bf16, tag="e")
                nc.scalar.activation(e[:], ps[:], Exp)
                nc.tensor.matmul(po0[0:65, :],
                                 vA[:, jt, h0 * 65:h0 * 65 + 65],
                                 e[:, 0:Q],
                                 start=(jt == 0), stop=(jt == NJT - 1))
                nc.tensor.matmul(po1[0:65, :],
                                 vA[:, jt, h1 * 65:h1 * 65 + 65],
                                 e[:, Q:2 * Q],
                                 start=(jt == 0), stop=(jt == NJT - 1))
            # denom rows live on psum partition 64; engines can't move
            # across partitions, so bounce via SBUF row 64 + DMA.
            for po, hh, dr in ((po0, h0, 0), (po1, h1, D)):
                den_t = den_pool.tile([P, Q], f32, tag="denrow")
                nc.scalar.copy(den_t[64:65, :], po[64:65, :])
                nc.sync.dma_start(den_dram[hh:hh + 1, :], den_t[64:65, :])
                nc.scalar.copy(aT[dr:dr + D, g, :], po[0:64, :])

            # normalize this pair by its softmax denominators (per-pair so
            # the chain overlaps later pairs' attention instead of
            # serializing at the end)
            den2 = den2_pool.tile([2, Q], f32, tag="den2")
            nc.sync.dma_start(den2[:], den_dram[h0:h1 + 1, :])
            rec2 = den2_pool.tile([2, Q], f32, tag="rec2")
            nc.vector.reciprocal(rec2[:], den2[:])
            nc.sync.dma_start(rec_dram[h0:h1 + 1, :], rec2[:])
            rbc = rbc_pool.tile([P, Q], f32, tag="rbc")
            nc.sync.dma_start(
                rbc[0:D, :], rec_dram[h0:h0 + 1, :].to_broadcast((D, Q)))
            nc.sync.dma_start(
                rbc[D:P, :], rec_dram[h1:h1 + 1, :].to_broadcast((D, Q)))
            nc.vector.tensor_tensor(aT[:, g, :], aT[:, g, :], rbc[:],
                                    mybir.AluOpType.mult)

        # ---- vocab projection ------------------------------------------
        for vc in range(NVC):
            wo_t = wo_pool.tile([P, ET, VCH], bf16, tag="wo")
            nc.sync.dma_start(wo_t[:], wo3[:, :, vc * VCH:(vc + 1) * VCH])
            bo_t = bo_pool.tile([P, VCH], bf16, tag="bo")
            nc.sync.dma_start(
                bo_t[:],
                bo[vc * VCH:(vc + 1) * VCH]
                .rearrange("(o v) -> o v", o=1).to_broadcast((P, VCH)))
            for tc4 in range(Q // P):
                ps = psP.tile([P, 1024], f32, tag="ps")
                pso = ps[:, 0:VCH]
                for et in range(ET):
                    nc.tensor.matmul(pso,
                                     aT[:, et, tc4 * P:(tc4 + 1) * P],
                                     wo_t[:, et, :],
                                     start=(et == 0), stop=(et == ET - 1))
                lt = lt_pool.tile([P, VCH], bf16, tag="lt")
                nc.vector.tensor_tensor(lt[:], pso, bo_t[:],
                                        mybir.AluOpType.add)
                nc.scalar.dma_start(
                    out[tc4 * P:(tc4 + 1) * P, vc * VCH:(vc + 1) * VCH],
                    lt[:])

    nc.compile()
    return nc


def get_nc():
    if "nc" not in _cache:
        _cache["nc"] = _build()
    return _cache["nc"]


def make_in_maps(x, Wq, bq, Wk, bk, Wv, bv, Wo, bo):
    import ml_dtypes

    def bf(a):
        return np.asarray(np.asarray(a, dtype=np.float32)
                          .astype(ml_dtypes.bfloat16))

    def f32a(a):
        return np.ascontiguousarray(np.asarray(a, dtype=np.float32))

    x = bf(x)
    Wq, Wk, Wv, Wo = bf(Wq), bf(Wk), bf(Wv), bf(Wo)
    bv, bo = bf(bv), bf(bo)
    bq, bk = f32a(bq), f32a(bk)
    in_maps = []
    for c in range(NCORES):
        b, qs = c // 4, (c % 4) * Q
        xbm = np.ascontiguousarray(np.roll(x[b], -qs, axis=0))
        in_maps.append({"xb": xbm, "wq": Wq, "wk": Wk, "wv": Wv, "wo": Wo,
                        "bq": bq, "bk": bk, "bv": bv, "bo": bo})
    return in_maps


def gather(results):
    out = np.empty((B, S, V), dtype=np.float32)
    for c in range(NCORES):
        b, qs = c // 4, (c % 4) * Q
        out[b, qs:qs + Q] = np.asarray(results[c]["out"],
                                       dtype=np.float32)
    return out


def kernel(**inputs):
    from concourse.bass_utils import run_bass_kernel_spmd

    nc = get_nc()
    in_maps = make_in_maps(**inputs)
    res = run_bass_kernel_spmd(nc, in_maps, list(range(NCORES)), trace=False)
    return gather(res.results)


# revision 22
# speedup vs baseline: 1.5672x; 1.0130x over previous
"""Multi-head self-attention + vocab projection, 8-core TRN2 Bass kernel.

Problem: x[2,2048,1024] -> logits[2,2048,32000]
  q/k/v = x@W{q,k,v}+b, 16 heads x 64; attn = softmax(qk^T/8)v; out = attn@Wo+bo

Sharding: data-parallel over the 4096 token rows -> 8 cores x 512 query rows
(cores 0-3 batch 0, cores 4-7 batch 1). Each core receives its full batch
(2048 tokens) for K/V, ROLLED so that its 512 query rows are rows 0:512 —
softmax is permutation-invariant over the kv axis, so rolling is safe and
makes the SPMD program core-id independent. Wo is column-streamed in full on
every core; logits are written with no cross-core reduce.

x and all weights are converted to bf16 host-side (halves HBM traffic for
the dominant Wo stream; matmuls accumulate in f32 psum). xT comes from XBAR
DMA-transpose — no PE transposes. Attention is computed transposed
(scoresT[j,q] = kT^T qT) so exp(scoresT) feeds attn@V directly as lhsT and
the attention output lands as attn_outT[emb, tok] — exactly the lhsT layout
the vocab projection needs. The softmax denominator comes from an appended
ones-column on V; normalization is applied after attn@V via reciprocal +
DRAM-broadcast + elementwise multiply.

Projections and attention are INTERLEAVED per head-pair (the PE otherwise
micro-idles waiting on ACT exp between score/attn matmuls, which keeps the
HAM clock gate cold at 1.2 GHz — measured 300+ us of K=4/8 throttling in the
phase-separated version). Head-pair scores share one 2-bank psum tile so exp
runs once per kv-tile over [128, 1024]. All SBUF pools are top-level so Wo
prefetch DMAs can run during the attention phase.
"""

import numpy as np

B, S, E = 2, 2048, 1024
H, D = 16, 64
V = 32000
P = 128
ET = E // P          # 8 embedding tiles
TOK = S              # kv tokens per core
Q = 512              # query rows per core
NJT = TOK // P       # 16 kv tiles
VCH = 500            # vocab chunk (psum bank = 512 f32)
NVC = V // VCH       # 64
DVC = 256            # v-projection dout chunk (= 4 heads)
NCORES = 8

_cache = {}


def _build():
    from contextlib import ExitStack

    import concourse.tile as tile
    from concourse import bacc, mybir

    f32 = mybir.dt.float32
    bf16 = mybir.dt.bfloat16
    Id = mybir.ActivationFunctionType.Identity
    Exp = mybir.ActivationFunctionType.Exp

    nc = bacc.Bacc("TRN2", target_bir_lowering=False, debug=False,
                   num_devices=NCORES)

    xb = nc.dram_tensor("xb", [TOK, E], bf16, kind="ExternalInput").ap()
    wq = nc.dram_tensor("wq", [E, E], bf16, kind="ExternalInput").ap()
    wk = nc.dram_tensor("wk", [E, E], bf16, kind="ExternalInput").ap()
    wv = nc.dram_tensor("wv", [E, E], bf16, kind="ExternalInput").ap()
    wo = nc.dram_tensor("wo", [E, V], bf16, kind="ExternalInput").ap()
    bq = nc.dram_tensor("bq", [E], f32, kind="ExternalInput").ap()
    bk = nc.dram_tensor("bk", [E], f32, kind="ExternalInput").ap()
    bv = nc.dram_tensor("bv", [E], bf16, kind="ExternalInput").ap()
    bo = nc.dram_tensor("bo", [V], bf16, kind="ExternalInput").ap()
    out = nc.dram_tensor("out", [Q, V], bf16, kind="ExternalOutput").ap()

    wq3 = wq.rearrange("(et p) d -> p et d", p=P)
    wk3 = wk.rearrange("(et p) d -> p et d", p=P)
    wv3 = wv.rearrange("(et p) d -> p et d", p=P)
    wo3 = wo.rearrange("(et p) v -> p et v", p=P)

    with tile.TileContext(nc) as tc, ExitStack() as ctx:
        # ---- pools (all top-level: scheduling is purely dep-driven) ----
        consts = ctx.enter_context(tc.tile_pool(name="consts", bufs=1))
        xT_pool = ctx.enter_context(tc.tile_pool(name="xT", bufs=1))
        kT_pool = ctx.enter_context(tc.tile_pool(name="kT", bufs=1))
        vA_pool = ctx.enter_context(tc.tile_pool(name="vA", bufs=1))
        qT_pool = ctx.enter_context(tc.tile_pool(name="qT", bufs=1))
        aT_pool = ctx.enter_context(tc.tile_pool(name="aT", bufs=1))
        dn_pool = ctx.enter_context(tc.tile_pool(name="dn", bufs=1))
        wqk_pool = ctx.enter_context(tc.tile_pool(name="wqk", bufs=2))
        wv_pool = ctx.enter_context(tc.tile_pool(name="wvp", bufs=2))
        e_pool = ctx.enter_context(tc.tile_pool(name="epool", bufs=4))
        den_pool = ctx.enter_context(tc.tile_pool(name="denrow", bufs=2))
        den2_pool = ctx.enter_context(tc.tile_pool(name="den2", bufs=1))
        rbc_pool = ctx.enter_context(tc.tile_pool(name="rbc", bufs=2))
        wo_pool = ctx.enter_context(tc.tile_pool(name="wo", bufs=5))
        lt_pool = ctx.enter_context(tc.tile_pool(name="lt", bufs=4))
        bo_pool = ctx.enter_context(tc.tile_pool(name="bo", bufs=3))
        dram_pool = ctx.enter_context(
            tc.tile_pool(name="dramscratch", bufs=1, space="DRAM"))
        # PSUM: shared 2-bank-slot pool (3 bufs) + attention accums (2x1
        # bank) = 8 banks exactly
        psP = ctx.enter_context(tc.tile_pool(name="ps", bufs=3,
                                             space="PSUM"))
        psA = ctx.enter_context(tc.tile_pool(name="psA", bufs=2,
                                             space="PSUM"))

        bq_sb = consts.tile([P, ET], f32)
        nc.sync.dma_start(bq_sb[:], bq.rearrange("(g p) -> p g", p=P))
        # fold the 1/sqrt(d) score scale into q: q' = (xWq + bq)/8
        nc.vector.tensor_scalar_mul(bq_sb[:], bq_sb[:], 0.125)
        bk_sb = consts.tile([P, ET], f32)
        nc.sync.dma_start(bk_sb[:], bk.rearrange("(g p) -> p g", p=P))
        bv_bc = consts.tile([P, E], bf16)
        nc.sync.dma_start(
            bv_bc[:],
            bv.rearrange("(o e) -> o e", o=1).to_broadcast((P, E)))

        xT = xT_pool.tile([P, ET, TOK], bf16)    # x^T, emb on partitions
        kT = kT_pool.tile([P, ET, TOK], bf16)    # k^T, d on partitions
        vA = vA_pool.tile([P, NJT, H * 65], bf16)  # v + ones col, per kv tile
        qT = qT_pool.tile([P, ET, Q], bf16)      # (q/8)^T
        aT = aT_pool.tile([P, ET, Q], bf16)      # attn_out^T
        den_dram = dram_pool.tile([H, Q], f32)
        rec_dram = dram_pool.tile([H, Q], f32)

        # ones column of vA (head-local column 64)
        vA4 = vA[:].rearrange("p j (h c) -> p j h c", c=65)
        nc.vector.memset(vA4[:, :, :, 64:65], 1.0)

        # prefetch the first weight tiles BEFORE the transposes so the
        # first projection matmuls aren't queued behind them
        wk0 = wqk_pool.tile([P, ET, P], bf16, tag="wqk")
        nc.sync.dma_start(wk0[:], wk3[:, :, 0:P])
        wv0 = wv_pool.tile([P, ET, DVC], bf16, tag="wv")
        nc.scalar.dma_start(wv0[:], wv3[:, :, 0:DVC])

        # x^T via XBAR dma-transpose, split across both HWDGE queues.
        # (Do NOT split these per token range: a partial-width transpose
        # destination produces wrong data on hardware — known xbar issue.)
        for et in range(ET):
            eng = nc.sync if et % 2 == 0 else nc.scalar
            eng.dma_start_transpose(xT[:, et, :],
                                    xb[:, et * P:(et + 1) * P])

        # ---- interleaved projections + attention, one head pair per g --
        for g in range(ET):
            # project kT[:, g, :] (dout tile g = heads 2g, 2g+1)
            if g == 0:
                wk_t = wk0
            else:
                wk_t = wqk_pool.tile([P, ET, P], bf16, tag="wqk")
                nc.sync.dma_start(wk_t[:], wk3[:, :, g * P:(g + 1) * P])
            for tcc in range(TOK // 512):
                ps = psP.tile([P, 1024], f32, tag="ps")
                for et in range(ET):
                    nc.tensor.matmul(ps[:, 0:512], wk_t[:, et, :],
                                     xT[:, et, tcc * 512:(tcc + 1) * 512],
                                     start=(et == 0), stop=(et == ET - 1))
                nc.vector.tensor_scalar_add(
                    kT[:, g, tcc * 512:(tcc + 1) * 512], ps[:, 0:512],
                    bk_sb[:, g:g + 1])
            # project qT[:, g, :] (score scale 1/8 folded in)
            wq_t = wqk_pool.tile([P, ET, P], bf16, tag="wqk")
            nc.sync.dma_start(wq_t[:], wq3[:, :, g * P:(g + 1) * P])
            ps = psP.tile([P, 1024], f32, tag="ps")
            for et in range(ET):
                nc.tensor.matmul(ps[:, 0:512], wq_t[:, et, :], xT[:, et, 0:Q],
                                 start=(et == 0), stop=(et == ET - 1))
            nc.vector.tensor_scalar(qT[:, g, :], ps[:, 0:512], 0.125,
                                    bq_sb[:, g:g + 1],
                                    mybir.AluOpType.mult,
                                    mybir.AluOpType.add)

            # project v chunk dvc=g//2 (heads 4*(g//2) .. +3) on even g
            if g % 2 == 0:
                dvc = g // 2
                if dvc == 0:
                    wv_t = wv0
                else:
                    wv_t = wv_pool.tile([P, ET, DVC], bf16, tag="wv")
                    nc.sync.dma_start(wv_t[:],
                                      wv3[:, :, dvc * DVC:(dvc + 1) * DVC])
                for tt in range(NJT):
                    ps = psP.tile([P, 1024], f32, tag="ps")
                    psv = ps[:, 0:DVC]
                    for et in range(ET):
                        nc.tensor.matmul(psv,
                                         xT[:, et, tt * P:(tt + 1) * P],
                                         wv_t[:, et, :],
                                         start=(et == 0),
                                         stop=(et == ET - 1))
                    h0 = dvc * (DVC // D)
                    dst = vA4[:, tt, h0:h0 + DVC // D, 0:64]
                    bvs = bv_bc[:, dvc * DVC:(dvc + 1) * DVC]
                    nc.vector.tensor_tensor(
                        dst, psv.rearrange("p (h c) -> p h c", c=D),
                        bvs.rearrange("p (h c) -> p h c", c=D),
                        mybir.AluOpType.add)

            # attention for heads 2g (rows 0:64) and 2g+1 (rows 64:128);
            # the two K=64 score matmuls pack into array row-halves via
            # tile_position and run concurrently.
            h0, h1 = 2 * g, 2 * g + 1
            po0 = psA.tile([P, Q], f32, tag="a")
            po1 = psA.tile([P, Q], f32, tag="a")
            for jt in range(NJT):
                ps = psP.tile([P, 2 * Q], f32, tag="ps")
                nc.tensor.matmul(ps[:, 0:Q],
                                 kT[0:D, g, jt * P:(jt + 1) * P],
                                 qT[0:D, g, :], start=True, stop=True,
                                 tile_position=(0, 0))
                nc.tensor.matmul(ps[:, Q:2 * Q],
                                 kT[D:P, g, jt * P:(jt + 1) * P],
                                 qT[D:P, g, :], start=True, stop=True,
                                 tile_position=(64, 0))
                e = e_pool.tile([P, 2 * Q], bf16, tag="e")
                nc.scalar.activation(e[:], ps[:], Exp)
                nc.tensor.matmul(po0[0:65, :],
                                 vA[:, jt, h0 * 65:h0 * 65 + 65],
                                 e[:, 0:Q],
                                 start=(jt == 0), stop=(jt == NJT - 1))
                nc.tensor.matmul(po1[0:65, :],
                                 vA[:, jt, h1 * 65:h1 * 65 + 65],
                                 e[:, Q:2 * Q],
                                 start=(jt == 0), stop=(jt == NJT - 1))
            # denom rows live on psum partition 64; engines can't move
            # across partitions, so bounce via SBUF row 64 + DMA.
            for po, hh, dr in ((po0, h0, 0), (po1, h1, D)):
                den_t = den_pool.tile([P, Q], f32, tag="denrow")
                nc.scalar.copy(den_t[64:65, :], po[64:65, :])
                nc.sync.dma_start(den_dram[hh:hh + 1, :], den_t[64:65, :])
                nc.scalar.copy(aT[dr:dr + D, g, :], po[0:64, :])

            # normalize this pair by its softmax denominators (per-pair so
            # the chain overlaps later pairs' attention instead of
            # serializing at the end)
            den2 = den2_pool.tile([2, Q], f32, tag="den2")
            nc.sync.dma_start(den2[:], den_dram[h0:h1 + 1, :])
            rec2 = den2_pool.tile([2, Q], f32, tag="rec2")
            nc.vector.reciprocal(rec2[:], den2[:])
            nc.sync.dma_start(rec_dram[h0:h1 + 1, :], rec2[:])
            rbc = rbc_pool.tile([P, Q], f32, tag="rbc")
            nc.sync.dma_start(
                rbc[0:D, :], rec_dram[h0:h0 + 1, :].to_broadcast((D, Q)))
            nc.sync.dma_start(
                rbc[D:P, :], rec_dram[h1:h1 + 1, :].to_broadcast((D, Q)))
            nc.vector.tensor_tensor(aT[:, g, :], aT[:, g, :], rbc[:],
                                    mybir.AluOpType.mult)

        # ---- vocab projection ------------------------------------------
        for vc in range(NVC):
            wo_t = wo_pool.tile([P, ET, VCH], bf16, tag="wo")
            nc.sync.dma_start(wo_t[:], wo3[:, :, vc * VCH:(vc + 1) * VCH])
            bo_t = bo_pool.tile([P, VCH], bf16, tag="bo")
            nc.sync.dma_start(
                bo_t[:],
                bo[vc * VCH:(vc + 1) * VCH]
                .rearrange("(o v) -> o v", o=1).to_broadcast((P, VCH)))
            for tc4 in range(Q // P):
                ps = psP.tile([P, 1024], f32, tag="ps")
                pso = ps[:, 0:VCH]
                for et in range(ET):
                    nc.tensor.matmul(pso,
                                     aT[:, et, tc4 * P:(tc4 + 1) * P],
                                     wo_t[:, et, :],
                                     start=(et == 0), stop=(et == ET - 1))
                lt = lt_pool.tile([P, VCH], bf16, tag="lt")
                nc.vector.tensor_tensor(lt[:], pso, bo_t[:],
                                        mybir.AluOpType.add)
                nc.scalar.dma_start(
                    out[tc4 * P:(tc4 + 1) * P, vc * VCH:(vc + 1) * VCH],
                    lt[:])

    nc.compile()
    return nc


def get_nc():
    if "nc" not in _cache:
        _cache["nc"] = _build()
    return _cache["nc"]


def make_in_maps(x, Wq, bq, Wk, bk, Wv, bv, Wo, bo):
    import ml_dtypes

    def bf(a):
        return np.asarray(np.asarray(a, dtype=np.float32)
                          .astype(ml_dtypes.bfloat16))

    def f32a(a):
        return np.ascontiguousarray(np.asarray(a, dtype=np.float32))

    x = bf(x)
    Wq, Wk, Wv, Wo = bf(Wq), bf(Wk), bf(Wv), bf(Wo)
    bv, bo = bf(bv), bf(bo)
    bq, bk = f32a(bq), f32a(bk)
    in_maps = []
    for c in range(NCORES):
        b, qs = c // 4, (c % 4) * Q
        xbm = np.ascontiguousarray(np.roll(x[b], -qs, axis=0))
        in_maps.append({"xb": xbm, "wq": Wq, "wk": Wk, "wv": Wv, "wo": Wo,
                        "bq": bq, "bk": bk, "bv": bv, "bo": bo})
    return in_maps


def gather(results):
    out = np.empty((B, S, V), dtype=np.float32)
    for c in range(NCORES):
        b, qs = c // 4, (c % 4) * Q
        out[b, qs:qs + Q] = np.asarray(results[c]["out"],
                                       dtype=np.float32)
    return out


def kernel(**inputs):
    from concourse.bass_utils import run_bass_kernel_spmd

    nc = get_nc()
    in_maps = make_in_maps(**inputs)
    res = run_bass_kernel_spmd(nc, in_maps, list(range(NCORES)), trace=False)
    return gather(res.results)


# revision 24
# speedup vs baseline: 1.6026x; 1.0226x over previous
"""Multi-head self-attention + vocab projection, 8-core TRN2 Bass kernel.

Problem: x[2,2048,1024] -> logits[2,2048,32000]
  q/k/v = x@W{q,k,v}+b, 16 heads x 64; attn = softmax(qk^T/8)v; out = attn@Wo+bo

Sharding: data-parallel over the 4096 token rows -> 8 cores x 512 query rows
(cores 0-3 batch 0, cores 4-7 batch 1). Each core receives its full batch
(2048 tokens) for K/V, ROLLED so that its 512 query rows are rows 0:512 —
softmax is permutation-invariant over the kv axis, so rolling is safe and
makes the SPMD program core-id independent. Wo is column-streamed in full on
every core; logits are written with no cross-core reduce.

x and all weights are converted to bf16 host-side (halves HBM traffic for
the dominant Wo stream; matmuls accumulate in f32 psum). xT comes from XBAR
DMA-transpose — no PE transposes. Attention is computed transposed
(scoresT[j,q] = kT^T qT) so exp(scoresT) feeds attn@V directly as lhsT and
the attention output lands as attn_outT[emb, tok] — exactly the lhsT layout
the vocab projection needs. The softmax denominator comes from an appended
ones-column on V; normalization is applied after attn@V via reciprocal +
DRAM-broadcast + elementwise multiply.

Projections and attention are INTERLEAVED per head-pair (the PE otherwise
micro-idles waiting on ACT exp between score/attn matmuls, which keeps the
HAM clock gate cold at 1.2 GHz — measured 300+ us of K=4/8 throttling in the
phase-separated version). Head-pair scores share one 2-bank psum tile so exp
runs once per kv-tile over [128, 1024]. All SBUF pools are top-level so Wo
prefetch DMAs can run during the attention phase.
"""

import numpy as np

B, S, E = 2, 2048, 1024
H, D = 16, 64
V = 32000
P = 128
ET = E // P          # 8 embedding tiles
TOK = S              # kv tokens per core
Q = 512              # query rows per core
NJT = TOK // P       # 16 kv tiles
VCH = 500            # vocab chunk (psum bank = 512 f32)
NVC = V // VCH       # 64
DVC = 256            # v-projection dout chunk (= 4 heads)
NCORES = 8

_cache = {}


def _build():
    from contextlib import ExitStack

    import concourse.tile as tile
    from concourse import bacc, mybir

    f32 = mybir.dt.float32
    bf16 = mybir.dt.bfloat16
    Id = mybir.ActivationFunctionType.Identity
    Exp = mybir.ActivationFunctionType.Exp

    nc = bacc.Bacc("TRN2", target_bir_lowering=False, debug=False,
                   num_devices=NCORES)

    xb = nc.dram_tensor("xb", [TOK, E], bf16, kind="ExternalInput").ap()
    wq = nc.dram_tensor("wq", [E, E], bf16, kind="ExternalInput").ap()
    wk = nc.dram_tensor("wk", [E, E], bf16, kind="ExternalInput").ap()
    wv = nc.dram_tensor("wv", [E, E], bf16, kind="ExternalInput").ap()
    wo = nc.dram_tensor("wo", [E, V], bf16, kind="ExternalInput").ap()
    bq = nc.dram_tensor("bq", [E], f32, kind="ExternalInput").ap()
    bk = nc.dram_tensor("bk", [E], f32, kind="ExternalInput").ap()
    bv = nc.dram_tensor("bv", [E], bf16, kind="ExternalInput").ap()
    bo = nc.dram_tensor("bo", [V], bf16, kind="ExternalInput").ap()
    out = nc.dram_tensor("out", [Q, V], bf16, kind="ExternalOutput").ap()

    wq3 = wq.rearrange("(et p) d -> p et d", p=P)
    wk3 = wk.rearrange("(et p) d -> p et d", p=P)
    wv3 = wv.rearrange("(et p) d -> p et d", p=P)
    wo3 = wo.rearrange("(et p) v -> p et v", p=P)

    with tile.TileContext(nc) as tc, ExitStack() as ctx:
        # ---- pools (all top-level: scheduling is purely dep-driven) ----
        consts = ctx.enter_context(tc.tile_pool(name="consts", bufs=1))
        xT_pool = ctx.enter_context(tc.tile_pool(name="xT", bufs=1))
        kT_pool = ctx.enter_context(tc.tile_pool(name="kT", bufs=1))
        vA_pool = ctx.enter_context(tc.tile_pool(name="vA", bufs=1))
        qT_pool = ctx.enter_context(tc.tile_pool(name="qT", bufs=1))
        aT_pool = ctx.enter_context(tc.tile_pool(name="aT", bufs=1))
        dn_pool = ctx.enter_context(tc.tile_pool(name="dn", bufs=1))
        wqk_pool = ctx.enter_context(tc.tile_pool(name="wqk", bufs=2))
        wv_pool = ctx.enter_context(tc.tile_pool(name="wvp", bufs=2))
        e_pool = ctx.enter_context(tc.tile_pool(name="epool", bufs=4))
        den_pool = ctx.enter_context(tc.tile_pool(name="denrow", bufs=2))
        den2_pool = ctx.enter_context(tc.tile_pool(name="den2", bufs=1))
        rbc_pool = ctx.enter_context(tc.tile_pool(name="rbc", bufs=2))
        wo_pool = ctx.enter_context(tc.tile_pool(name="wo", bufs=5))
        lt_pool = ctx.enter_context(tc.tile_pool(name="lt", bufs=4))
        bo_pool = ctx.enter_context(tc.tile_pool(name="bo", bufs=3))
        dram_pool = ctx.enter_context(
            tc.tile_pool(name="dramscratch", bufs=1, space="DRAM"))
        # PSUM: shared 2-bank-slot pool (3 bufs) + attention accums (2x1
        # bank) = 8 banks exactly
        psP = ctx.enter_context(tc.tile_pool(name="ps", bufs=3,
                                             space="PSUM"))
        psA = ctx.enter_context(tc.tile_pool(name="psA", bufs=2,
                                             space="PSUM"))

        bq_sb = consts.tile([P, ET], f32)
        nc.sync.dma_start(bq_sb[:], bq.rearrange("(g p) -> p g", p=P))
        # fold the 1/sqrt(d) score scale into q: q' = (xWq + bq)/8
        nc.vector.tensor_scalar_mul(bq_sb[:], bq_sb[:], 0.125)
        bk_sb = consts.tile([P, ET], f32)
        nc.sync.dma_start(bk_sb[:], bk.rearrange("(g p) -> p g", p=P))
        bv_bc = consts.tile([P, E], bf16)
        nc.sync.dma_start(
            bv_bc[:],
            bv.rearrange("(o e) -> o e", o=1).to_broadcast((P, E)))

        xT = xT_pool.tile([P, ET, TOK], bf16)    # x^T, emb on partitions
        kT = kT_pool.tile([P, ET, TOK], bf16)    # k^T, d on partitions
        vA = vA_pool.tile([P, NJT, H * 65], bf16)  # v + ones col, per kv tile
        qT = qT_pool.tile([P, ET, Q], bf16)      # (q/8)^T
        aT = aT_pool.tile([P, ET, Q], bf16)      # attn_out^T
        den_dram = dram_pool.tile([H, Q], f32)
        rec_dram = dram_pool.tile([H, Q], f32)

        # ones column of vA (head-local column 64); on GpSimd so the DVE
        # queue stays clear for the first projection-bias drains
        vA4 = vA[:].rearrange("p j (h c) -> p j h c", c=65)
        nc.gpsimd.memset(vA4[:, :, :, 64:65], 1.0)

        # prefetch the first weight tiles BEFORE the transposes so the
        # first projection matmuls aren't queued behind them
        wk0 = wqk_pool.tile([P, ET, P], bf16, tag="wqk")
        nc.sync.dma_start(wk0[:], wk3[:, :, 0:P])
        wv0 = wv_pool.tile([P, ET, DVC], bf16, tag="wv")
        nc.scalar.dma_start(wv0[:], wv3[:, :, 0:DVC])

        # x^T via XBAR dma-transpose, split across both HWDGE queues.
        # (Do NOT split these per token range: a partial-width transpose
        # destination produces wrong data on hardware — known xbar issue.)
        for et in range(ET):
            eng = nc.sync if et % 2 == 0 else nc.scalar
            eng.dma_start_transpose(xT[:, et, :],
                                    xb[:, et * P:(et + 1) * P])

        # ---- interleaved projections + attention, one head pair per g --
        for g in range(ET):
            # project kT[:, g, :] (dout tile g = heads 2g, 2g+1)
            if g == 0:
                wk_t = wk0
            else:
                wk_t = wqk_pool.tile([P, ET, P], bf16, tag="wqk")
                nc.sync.dma_start(wk_t[:], wk3[:, :, g * P:(g + 1) * P])
            for tcc in range(TOK // 512):
                ps = psP.tile([P, 1024], f32, tag="ps")
                for et in range(ET):
                    nc.tensor.matmul(ps[:, 0:512], wk_t[:, et, :],
                                     xT[:, et, tcc * 512:(tcc + 1) * 512],
                                     start=(et == 0), stop=(et == ET - 1))
                nc.vector.tensor_scalar_add(
                    kT[:, g, tcc * 512:(tcc + 1) * 512], ps[:, 0:512],
                    bk_sb[:, g:g + 1])
            # project qT[:, g, :] (score scale 1/8 folded in)
            wq_t = wqk_pool.tile([P, ET, P], bf16, tag="wqk")
            nc.sync.dma_start(wq_t[:], wq3[:, :, g * P:(g + 1) * P])
            ps = psP.tile([P, 1024], f32, tag="ps")
            for et in range(ET):
                nc.tensor.matmul(ps[:, 0:512], wq_t[:, et, :], xT[:, et, 0:Q],
                                 start=(et == 0), stop=(et == ET - 1))
            nc.vector.tensor_scalar(qT[:, g, :], ps[:, 0:512], 0.125,
                                    bq_sb[:, g:g + 1],
                                    mybir.AluOpType.mult,
                                    mybir.AluOpType.add)

            # project v chunk dvc=g//2 (heads 4*(g//2) .. +3) on even g
            if g % 2 == 0:
                dvc = g // 2
                if dvc == 0:
                    wv_t = wv0
                else:
                    wv_t = wv_pool.tile([P, ET, DVC], bf16, tag="wv")
                    nc.sync.dma_start(wv_t[:],
                                      wv3[:, :, dvc * DVC:(dvc + 1) * DVC])
                for tt in range(NJT):
                    ps = psP.tile([P, 1024], f32, tag="ps")
                    psv = ps[:, 0:DVC]
                    for et in range(ET):
                        nc.tensor.matmul(psv,
                                         xT[:, et, tt * P:(tt + 1) * P],
                                         wv_t[:, et, :],
                                         start=(et == 0),
                                         stop=(et == ET - 1))
                    h0 = dvc * (DVC // D)
                    dst = vA4[:, tt, h0:h0 + DVC // D, 0:64]
                    bvs = bv_bc[:, dvc * DVC:(dvc + 1) * DVC]
                    nc.vector.tensor_tensor(
                        dst, psv.rearrange("p (h c) -> p h c", c=D),
                        bvs.rearrange("p (h c) -> p h c", c=D),
                        mybir.AluOpType.add)

            # attention for heads 2g (rows 0:64) and 2g+1 (rows 64:128);
            # the two K=64 score matmuls pack into array row-halves via
            # tile_position and run concurrently.
            h0, h1 = 2 * g, 2 * g + 1
            po0 = psA.tile([P, Q], f32, tag="a")
            po1 = psA.tile([P, Q], f32, tag="a")
            for jt in range(NJT):
                ps = psP.tile([P, 2 * Q], f32, tag="ps")
                nc.tensor.matmul(ps[:, 0:Q],
                                 kT[0:D, g, jt * P:(jt + 1) * P],
                                 qT[0:D, g, :], start=True, stop=True,
                                 tile_position=(0, 0))
                nc.tensor.matmul(ps[:, Q:2 * Q],
                                 kT[D:P, g, jt * P:(jt + 1) * P],
                                 qT[D:P, g, :], start=True, stop=True,
                                 tile_position=(64, 0))
                e = e_pool.tile([P, 2 * Q], bf16, tag="e")
                nc.scalar.activation(e[:], ps[:], Exp)
                nc.tensor.matmul(po0[0:65, :],
                                 vA[:, jt, h0 * 65:h0 * 65 + 65],
                                 e[:, 0:Q],
                                 start=(jt == 0), stop=(jt == NJT - 1))
                nc.tensor.matmul(po1[0:65, :],
                                 vA[:, jt, h1 * 65:h1 * 65 + 65],
                                 e[:, Q:2 * Q],
                                 start=(jt == 0), stop=(jt == NJT - 1))
            # denom rows live on psum partition 64; engines can't move
            # across partitions, so bounce via SBUF row 64 + DMA. These
            # drains run on DVE: on ACT they'd queue behind the exps and
            # delay the psum-accumulator release for the next pair.
            for po, hh, dr in ((po0, h0, 0), (po1, h1, D)):
                den_t = den_pool.tile([P, Q], f32, tag="denrow")
                nc.vector.tensor_copy(den_t[64:65, :], po[64:65, :])
                nc.sync.dma_start(den_dram[hh:hh + 1, :], den_t[64:65, :])
                nc.vector.tensor_copy(aT[dr:dr + D, g, :], po[0:64, :])

            # normalize this pair by its softmax denominators (per-pair so
            # the chain overlaps later pairs' attention instead of
            # serializing at the end)
            den2 = den2_pool.tile([2, Q], f32, tag="den2")
            nc.sync.dma_start(den2[:], den_dram[h0:h1 + 1, :])
            rec2 = den2_pool.tile([2, Q], f32, tag="rec2")
            nc.vector.reciprocal(rec2[:], den2[:])
            nc.sync.dma_start(rec_dram[h0:h1 + 1, :], rec2[:])
            rbc = rbc_pool.tile([P, Q], f32, tag="rbc")
            nc.sync.dma_start(
                rbc[0:D, :], rec_dram[h0:h0 + 1, :].to_broadcast((D, Q)))
            nc.sync.dma_start(
                rbc[D:P, :], rec_dram[h1:h1 + 1, :].to_broadcast((D, Q)))
            nc.vector.tensor_tensor(aT[:, g, :], aT[:, g, :], rbc[:],
                                    mybir.AluOpType.mult)

        # ---- vocab projection ------------------------------------------
        for vc in range(NVC):
            wo_t = wo_pool.tile([P, ET, VCH], bf16, tag="wo")
            nc.sync.dma_start(wo_t[:], wo3[:, :, vc * VCH:(vc + 1) * VCH])
            bo_t = bo_pool.tile([P, VCH], bf16, tag="bo")
            nc.sync.dma_start(
                bo_t[:],
                bo[vc * VCH:(vc + 1) * VCH]
                .rearrange("(o v) -> o v", o=1).to_broadcast((P, VCH)))
            for tc4 in range(Q // P):
                ps = psP.tile([P, 1024], f32, tag="ps")
                pso = ps[:, 0:VCH]
                for et in range(ET):
                    nc.tensor.matmul(pso,
                                     aT[:, et, tc4 * P:(tc4 + 1) * P],
                                     wo_t[:, et, :],
                                     start=(et == 0), stop=(et == ET - 1))
                lt = lt_pool.tile([P, VCH], bf16, tag="lt")
                nc.vector.tensor_tensor(lt[:], pso, bo_t[:],
                                        mybir.AluOpType.add)
                nc.scalar.dma_start(
                    out[tc4 * P:(tc4 + 1) * P, vc * VCH:(vc + 1) * VCH],
                    lt[:])

    nc.compile()
    return nc


def get_nc():
    if "nc" not in _cache:
        _cache["nc"] = _build()
    return _cache["nc"]


def make_in_maps(x, Wq, bq, Wk, bk, Wv, bv, Wo, bo):
    import ml_dtypes

    def bf(a):
        return np.asarray(np.asarray(a, dtype=np.float32)
                          .astype(ml_dtypes.bfloat16))

    def f32a(a):
        return np.ascontiguousarray(np.asarray(a, dtype=np.float32))

    x = bf(x)
    Wq, Wk, Wv, Wo = bf(Wq), bf(Wk), bf(Wv), bf(Wo)
    bv, bo = bf(bv), bf(bo)
    bq, bk = f32a(bq), f32a(bk)
    in_maps = []
    for c in range(NCORES):
        b, qs = c // 4, (c % 4) * Q
        xbm = np.ascontiguousarray(np.roll(x[b], -qs, axis=0))
        in_maps.append({"xb": xbm, "wq": Wq, "wk": Wk, "wv": Wv, "wo": Wo,
                        "bq": bq, "bk": bk, "bv": bv, "bo": bo})
    return in_maps


def gather(results):
    out = np.empty((B, S, V), dtype=np.float32)
    for c in range(NCORES):
        b, qs = c // 4, (c % 4) * Q
        out[b, qs:qs + Q] = np.asarray(results[c]["out"],
                                       dtype=np.float32)
    return out


def kernel(**inputs):
    from concourse.bass_utils import run_bass_kernel_spmd

    nc = get_nc()
    in_maps = make_in_maps(**inputs)
    res = run_bass_kernel_spmd(nc, in_maps, list(range(NCORES)), trace=False)
    return gather(res.results)


# revision 26
# speedup vs baseline: 1.6245x; 1.0136x over previous
"""Multi-head self-attention + vocab projection, 8-core TRN2 Bass kernel.

Problem: x[2,2048,1024] -> logits[2,2048,32000]
  q/k/v = x@W{q,k,v}+b, 16 heads x 64; attn = softmax(qk^T/8)v; out = attn@Wo+bo

Sharding: data-parallel over the 4096 token rows -> 8 cores x 512 query rows
(cores 0-3 batch 0, cores 4-7 batch 1). Each core receives its full batch
(2048 tokens) for K/V, ROLLED so that its 512 query rows are rows 0:512 —
softmax is permutation-invariant over the kv axis, so rolling is safe and
makes the SPMD program core-id independent. Wo is column-streamed in full on
every core; logits are written with no cross-core reduce.

x and all weights are converted to bf16 host-side (halves HBM traffic for
the dominant Wo stream; matmuls accumulate in f32 psum). xT comes from XBAR
DMA-transpose — no PE transposes. Attention is computed transposed
(scoresT[j,q] = kT^T qT) so exp(scoresT) feeds attn@V directly as lhsT and
the attention output lands as attn_outT[emb, tok] — exactly the lhsT layout
the vocab projection needs. The softmax denominator comes from an appended
ones-column on V; normalization is applied after attn@V via reciprocal +
DRAM-broadcast + elementwise multiply.

Projections and attention are INTERLEAVED per head-pair (the PE otherwise
micro-idles waiting on ACT exp between score/attn matmuls, which keeps the
HAM clock gate cold at 1.2 GHz — measured 300+ us of K=4/8 throttling in the
phase-separated version). Head-pair scores share one 2-bank psum tile so exp
runs once per kv-tile over [128, 1024]. All SBUF pools are top-level so Wo
prefetch DMAs can run during the attention phase.
"""

import numpy as np

B, S, E = 2, 2048, 1024
H, D = 16, 64
V = 32000
P = 128
ET = E // P          # 8 embedding tiles
TOK = S              # kv tokens per core
Q = 512              # query rows per core
NJT = TOK // P       # 16 kv tiles
VCH = 500            # vocab chunk (psum bank = 512 f32)
NVC = V // VCH       # 64
DVC = 256            # v-projection dout chunk (= 4 heads)
NCORES = 8

_cache = {}


def _build():
    from contextlib import ExitStack

    import concourse.tile as tile
    from concourse import bacc, mybir

    f32 = mybir.dt.float32
    bf16 = mybir.dt.bfloat16
    Id = mybir.ActivationFunctionType.Identity
    Exp = mybir.ActivationFunctionType.Exp

    nc = bacc.Bacc("TRN2", target_bir_lowering=False, debug=False,
                   num_devices=NCORES)

    xb = nc.dram_tensor("xb", [TOK, E], bf16, kind="ExternalInput").ap()
    wq = nc.dram_tensor("wq", [E, E], bf16, kind="ExternalInput").ap()
    wk = nc.dram_tensor("wk", [E, E], bf16, kind="ExternalInput").ap()
    wv = nc.dram_tensor("wv", [E, E], bf16, kind="ExternalInput").ap()
    wo = nc.dram_tensor("wo", [E, V], bf16, kind="ExternalInput").ap()
    bq = nc.dram_tensor("bq", [E], f32, kind="ExternalInput").ap()
    bk = nc.dram_tensor("bk", [E], f32, kind="ExternalInput").ap()
    bv = nc.dram_tensor("bv", [E], bf16, kind="ExternalInput").ap()
    bo = nc.dram_tensor("bo", [V], bf16, kind="ExternalInput").ap()
    out = nc.dram_tensor("out", [Q, V], bf16, kind="ExternalOutput").ap()

    wq3 = wq.rearrange("(et p) d -> p et d", p=P)
    wk3 = wk.rearrange("(et p) d -> p et d", p=P)
    wv3 = wv.rearrange("(et p) d -> p et d", p=P)
    wo3 = wo.rearrange("(et p) v -> p et v", p=P)

    with tile.TileContext(nc) as tc, ExitStack() as ctx:
        # ---- pools (all top-level: scheduling is purely dep-driven) ----
        consts = ctx.enter_context(tc.tile_pool(name="consts", bufs=1))
        xT_pool = ctx.enter_context(tc.tile_pool(name="xT", bufs=1))
        kT_pool = ctx.enter_context(tc.tile_pool(name="kT", bufs=1))
        vA_pool = ctx.enter_context(tc.tile_pool(name="vA", bufs=1))
        qT_pool = ctx.enter_context(tc.tile_pool(name="qT", bufs=1))
        aT_pool = ctx.enter_context(tc.tile_pool(name="aT", bufs=1))
        dn_pool = ctx.enter_context(tc.tile_pool(name="dn", bufs=1))
        wqk_pool = ctx.enter_context(tc.tile_pool(name="wqk", bufs=3))
        wv_pool = ctx.enter_context(tc.tile_pool(name="wvp", bufs=2))
        e_pool = ctx.enter_context(tc.tile_pool(name="epool", bufs=4))
        den_pool = ctx.enter_context(tc.tile_pool(name="denrow", bufs=2))
        den2_pool = ctx.enter_context(tc.tile_pool(name="den2", bufs=1))
        rbc_pool = ctx.enter_context(tc.tile_pool(name="rbc", bufs=2))
        wo_pool = ctx.enter_context(tc.tile_pool(name="wo", bufs=5))
        lt_pool = ctx.enter_context(tc.tile_pool(name="lt", bufs=4))
        bo_pool = ctx.enter_context(tc.tile_pool(name="bo", bufs=3))
        dram_pool = ctx.enter_context(
            tc.tile_pool(name="dramscratch", bufs=1, space="DRAM"))
        # PSUM: shared 2-bank-slot pool (3 bufs) + attention accums (2x1
        # bank) = 8 banks exactly
        psP = ctx.enter_context(tc.tile_pool(name="ps", bufs=3,
                                             space="PSUM"))
        psA = ctx.enter_context(tc.tile_pool(name="psA", bufs=2,
                                             space="PSUM"))

        bq_sb = consts.tile([P, ET], f32)
        nc.sync.dma_start(bq_sb[:], bq.rearrange("(g p) -> p g", p=P))
        # fold the 1/sqrt(d) score scale into q: q' = (xWq + bq)/8
        nc.vector.tensor_scalar_mul(bq_sb[:], bq_sb[:], 0.125)
        bk_sb = consts.tile([P, ET], f32)
        nc.sync.dma_start(bk_sb[:], bk.rearrange("(g p) -> p g", p=P))
        bv_bc = consts.tile([P, E], bf16)
        nc.sync.dma_start(
            bv_bc[:],
            bv.rearrange("(o e) -> o e", o=1).to_broadcast((P, E)))

        xT = xT_pool.tile([P, ET, TOK], bf16)    # x^T, emb on partitions
        kT = kT_pool.tile([P, ET, TOK], bf16)    # k^T, d on partitions
        vA = vA_pool.tile([P, NJT, H * 65], bf16)  # v + ones col, per kv tile
        qT = qT_pool.tile([P, ET, Q], bf16)      # (q/8)^T
        aT = aT_pool.tile([P, ET, Q], bf16)      # attn_out^T
        den_dram = dram_pool.tile([H, Q], f32)
        rec_dram = dram_pool.tile([H, Q], f32)

        # ones column of vA (head-local column 64); on GpSimd so the DVE
        # queue stays clear for the first projection-bias drains
        vA4 = vA[:].rearrange("p j (h c) -> p j h c", c=65)
        nc.gpsimd.memset(vA4[:, :, :, 64:65], 1.0)

        # prefetch the first weight tiles BEFORE the transposes so the
        # first projection matmuls aren't queued behind them
        wk0 = wqk_pool.tile([P, ET, P], bf16, tag="wqk")
        nc.sync.dma_start(wk0[:], wk3[:, :, 0:P])
        wv0 = wv_pool.tile([P, ET, DVC], bf16, tag="wv")
        nc.scalar.dma_start(wv0[:], wv3[:, :, 0:DVC])

        # x^T via XBAR dma-transpose, split across both HWDGE queues.
        # (Do NOT split these per token range: a partial-width transpose
        # destination produces wrong data on hardware — known xbar issue.)
        for et in range(ET):
            eng = nc.sync if et % 2 == 0 else nc.scalar
            eng.dma_start_transpose(xT[:, et, :],
                                    xb[:, et * P:(et + 1) * P])

        # ---- interleaved projections + attention -----------------------
        def proj_kq(g, wk_t, wq_t):
            # kT[:, g, :] (dout tile g = heads 2g, 2g+1)
            for tcc in range(TOK // 512):
                ps = psP.tile([P, 1024], f32, tag="ps")
                for et in range(ET):
                    nc.tensor.matmul(ps[:, 0:512], wk_t[:, et, :],
                                     xT[:, et, tcc * 512:(tcc + 1) * 512],
                                     start=(et == 0), stop=(et == ET - 1))
                nc.vector.tensor_scalar_add(
                    kT[:, g, tcc * 512:(tcc + 1) * 512], ps[:, 0:512],
                    bk_sb[:, g:g + 1])
            # qT[:, g, :] (score scale 1/8 folded in)
            ps = psP.tile([P, 1024], f32, tag="ps")
            for et in range(ET):
                nc.tensor.matmul(ps[:, 0:512], wq_t[:, et, :],
                                 xT[:, et, 0:Q],
                                 start=(et == 0), stop=(et == ET - 1))
            nc.vector.tensor_scalar(qT[:, g, :], ps[:, 0:512], 0.125,
                                    bq_sb[:, g:g + 1],
                                    mybir.AluOpType.mult,
                                    mybir.AluOpType.add)

        def proj_v(dvc, wv_t):
            for tt in range(NJT):
                ps = psP.tile([P, 1024], f32, tag="ps")
                psv = ps[:, 0:DVC]
                for et in range(ET):
                    nc.tensor.matmul(psv,
                                     xT[:, et, tt * P:(tt + 1) * P],
                                     wv_t[:, et, :],
                                     start=(et == 0), stop=(et == ET - 1))
                h0 = dvc * (DVC // D)
                dst = vA4[:, tt, h0:h0 + DVC // D, 0:64]
                bvs = bv_bc[:, dvc * DVC:(dvc + 1) * DVC]
                nc.vector.tensor_tensor(
                    dst, psv.rearrange("p (h c) -> p h c", c=D),
                    bvs.rearrange("p (h c) -> p h c", c=D),
                    mybir.AluOpType.add)

        def load_wk(g):
            t = wqk_pool.tile([P, ET, P], bf16, tag="wqk")
            nc.sync.dma_start(t[:], wk3[:, :, g * P:(g + 1) * P])
            return t

        def load_wq(g):
            t = wqk_pool.tile([P, ET, P], bf16, tag="wqk")
            nc.sync.dma_start(t[:], wq3[:, :, g * P:(g + 1) * P])
            return t

        def load_wv(dvc):
            t = wv_pool.tile([P, ET, DVC], bf16, tag="wv")
            nc.sync.dma_start(t[:], wv3[:, :, dvc * DVC:(dvc + 1) * DVC])
            return t

        # prologue: pair 0 + v chunk 0, then run one pair of projections
        # AHEAD of the attention that consumes them — the projection psum
        # drains (DVE) then complete during the previous pair's attention
        # instead of stalling the scores.
        proj_kq(0, wk0, load_wq(0))
        proj_v(0, wv0)
        for g in range(ET):
            if g + 1 < ET:
                proj_kq(g + 1, load_wk(g + 1), load_wq(g + 1))
                if (g + 1) % 2 == 0:
                    proj_v((g + 1) // 2, load_wv((g + 1) // 2))

            # attention for heads 2g (rows 0:64) and 2g+1 (rows 64:128);
            # the two K=64 score matmuls pack into array row-halves via
            # tile_position and run concurrently.
            h0, h1 = 2 * g, 2 * g + 1
            po0 = psA.tile([P, Q], f32, tag="a")
            po1 = psA.tile([P, Q], f32, tag="a")
            for jt in range(NJT):
                ps = psP.tile([P, 2 * Q], f32, tag="ps")
                nc.tensor.matmul(ps[:, 0:Q],
                                 kT[0:D, g, jt * P:(jt + 1) * P],
                                 qT[0:D, g, :], start=True, stop=True,
                                 tile_position=(0, 0))
                nc.tensor.matmul(ps[:, Q:2 * Q],
                                 kT[D:P, g, jt * P:(jt + 1) * P],
                                 qT[D:P, g, :], start=True, stop=True,
                                 tile_position=(64, 0))
                e = e_pool.tile([P, 2 * Q], bf16, tag="e")
                nc.scalar.activation(e[:], ps[:], Exp)
                nc.tensor.matmul(po0[0:65, :],
                                 vA[:, jt, h0 * 65:h0 * 65 + 65],
                                 e[:, 0:Q],
                                 start=(jt == 0), stop=(jt == NJT - 1))
                nc.tensor.matmul(po1[0:65, :],
                                 vA[:, jt, h1 * 65:h1 * 65 + 65],
                                 e[:, Q:2 * Q],
                                 start=(jt == 0), stop=(jt == NJT - 1))
            # denom rows live on psum partition 64; engines can't move
            # across partitions, so bounce via SBUF row 64 + DMA. These
            # drains run on DVE: on ACT they'd queue behind the exps and
            # delay the psum-accumulator release for the next pair.
            for po, hh, dr in ((po0, h0, 0), (po1, h1, D)):
                den_t = den_pool.tile([P, Q], f32, tag="denrow")
                nc.vector.tensor_copy(den_t[64:65, :], po[64:65, :])
                nc.sync.dma_start(den_dram[hh:hh + 1, :], den_t[64:65, :])
                nc.vector.tensor_copy(aT[dr:dr + D, g, :], po[0:64, :])

            # normalize this pair by its softmax denominators (per-pair so
            # the chain overlaps later pairs' attention instead of
            # serializing at the end)
            den2 = den2_pool.tile([2, Q], f32, tag="den2")
            nc.sync.dma_start(den2[:], den_dram[h0:h1 + 1, :])
            rec2 = den2_pool.tile([2, Q], f32, tag="rec2")
            nc.vector.reciprocal(rec2[:], den2[:])
            nc.sync.dma_start(rec_dram[h0:h1 + 1, :], rec2[:])
            rbc = rbc_pool.tile([P, Q], f32, tag="rbc")
            nc.sync.dma_start(
                rbc[0:D, :], rec_dram[h0:h0 + 1, :].to_broadcast((D, Q)))
            nc.sync.dma_start(
                rbc[D:P, :], rec_dram[h1:h1 + 1, :].to_broadcast((D, Q)))
            nc.vector.tensor_tensor(aT[:, g, :], aT[:, g, :], rbc[:],
                                    mybir.AluOpType.mult)

        # ---- vocab projection ------------------------------------------
        for vc in range(NVC):
            wo_t = wo_pool.tile([P, ET, VCH], bf16, tag="wo")
            nc.sync.dma_start(wo_t[:], wo3[:, :, vc * VCH:(vc + 1) * VCH])
            bo_t = bo_pool.tile([P, VCH], bf16, tag="bo")
            nc.sync.dma_start(
                bo_t[:],
                bo[vc * VCH:(vc + 1) * VCH]
                .rearrange("(o v) -> o v", o=1).to_broadcast((P, VCH)))
            for tc4 in range(Q // P):
                ps = psP.tile([P, 1024], f32, tag="ps")
                pso = ps[:, 0:VCH]
                for et in range(ET):
                    nc.tensor.matmul(pso,
                                     aT[:, et, tc4 * P:(tc4 + 1) * P],
                                     wo_t[:, et, :],
                                     start=(et == 0), stop=(et == ET - 1))
                lt = lt_pool.tile([P, VCH], bf16, tag="lt")
                nc.vector.tensor_tensor(lt[:], pso, bo_t[:],
                                        mybir.AluOpType.add)
                nc.scalar.dma_start(
                    out[tc4 * P:(tc4 + 1) * P, vc * VCH:(vc + 1) * VCH],
                    lt[:])

    nc.compile()
    return nc


def get_nc():
    if "nc" not in _cache:
        _cache["nc"] = _build()
    return _cache["nc"]


def make_in_maps(x, Wq, bq, Wk, bk, Wv, bv, Wo, bo):
    import ml_dtypes

    def bf(a):
        return np.asarray(np.asarray(a, dtype=np.float32)
                          .astype(ml_dtypes.bfloat16))

    def f32a(a):
        return np.ascontiguousarray(np.asarray(a, dtype=np.float32))

    x = bf(x)
    Wq, Wk, Wv, Wo = bf(Wq), bf(Wk), bf(Wv), bf(Wo)
    bv, bo = bf(bv), bf(bo)
    bq, bk = f32a(bq), f32a(bk)
    in_maps = []
    for c in range(NCORES):
        b, qs = c // 4, (c % 4) * Q
        xbm = np.ascontiguousarray(np.roll(x[b], -qs, axis=0))
        in_maps.append({"xb": xbm, "wq": Wq, "wk": Wk, "wv": Wv, "wo": Wo,
                        "bq": bq, "bk": bk, "bv": bv, "bo": bo})
    return in_maps


def gather(results):
    out = np.empty((B, S, V), dtype=np.float32)
    for c in range(NCORES):
        b, qs = c // 4, (c % 4) * Q
        out[b, qs:qs + Q] = np.asarray(results[c]["out"],
                                       dtype=np.float32)
    return out


def kernel(**inputs):
    from concourse.bass_utils import run_bass_kernel_spmd

    nc = get_nc()
    in_maps = make_in_maps(**inputs)
    res = run_bass_kernel_spmd(nc, in_maps, list(range(NCORES)), trace=False)
    return gather(res.results)
